# revision 1
# baseline (speedup 1.0000x reference)
"""3-layer GCN (PyG GCNConv-style) Bass/Trainium2 kernel, 8-way SPMD.

Strategy (standard 1D graph partitioning, dst-sharded):
  - Core c owns node rows [c*6250, (c+1)*6250).
  - Per layer: local GEMM H = X@W (PE-transposed activations, W as moving
    operand), scaled by dinv -> Htilde; one AllGather -> full table [50000,F];
    gathers read it as two 25000-row views so indices fit int16.
  - Message passing: dst-sorted edges, bulk dma_gather of source rows,
    aggregated per 128-node window with one-hot "selection" matmuls
    (S[e, n] = 1 iff dst_local[e]==n) accumulating in PSUM; self-loop added
    via identity matmul; out = relu(dinv * psum). S matrices are built 4 per
    DVE op (batched is_equal against broadcast dst_local columns).
  - Schedule (runs per (window, src-half) padded to the max over cores) is
    identical on all cores -> single NEFF; per-core data lives in the
    gather-index / dst-local metadata input tensors.
  - Finals: z kept in SBUF, global sum via AllReduce, z/sum -> tanh^2 ->
    row L2 normalize, all column-batched.
"""

import numpy as np

# ---- problem constants (hardcoded per contest contract) ----
N = 50000
F0, F1, F2, F3 = 512, 512, 256, 128
NCORES = 8
OWN = N // NCORES            # 6250 rows per core
WIN = 128
NW = (OWN + WIN - 1) // WIN  # 49 windows
OWN_PAD = NW * WIN           # 6272
HALFR = OWN // 2             # 3125: per-rank row split for the 2-chunk AllGather
TBL = NCORES * HALFR         # 25000 rows per gather table (< 32768, int16 ok)
GL = (8, 8, 8)               # gather tiles per dma_gather call, per layer
CALL = max(GL) * 128         # stream padding granularity (covers all layers)
SENT = 65000.0               # dst_local sentinel -> never matches iota 0..127
EPS = 1e-12

_BUILD_CACHE = {}


# --------------------------------------------------------------------------
# host-side schedule construction (pure index bookkeeping)
# --------------------------------------------------------------------------

def _build_schedule(src, dst):
    """Returns (sched, per_core) where sched is core-independent."""
    src = src.astype(np.int64)
    dst = dst.astype(np.int64)
    core = dst // OWN
    win = (dst % OWN) // WIN
    r = src % OWN
    chunk = (r >= HALFR).astype(np.int64)

    key = (core * NW + win) * 2 + chunk
    order = np.argsort(key, kind="stable")
    counts = np.bincount(key, minlength=NCORES * NW * 2).reshape(NCORES, NW, 2)
    R = counts.max(axis=0)                      # [NW, 2] padded run lengths
    pos = np.zeros((NW, 2), np.int64)           # start position of run (w,c)
    pos[1:, 0] = np.cumsum(R[:-1, 0])
    pos[1:, 1] = np.cumsum(R[:-1, 1])
    slen = R.sum(axis=0)                        # [2] stream lengths
    L = ((slen + CALL - 1) // CALL) * CALL      # padded to gather-call multiple

    # window-of-position per stream (pads extend each run; tail -> -1)
    wof = []
    for c in (0, 1):
        a = np.full(L[c], -1, np.int64)
        a[: slen[c]] = np.repeat(np.arange(NW), R[:, c])
        wof.append(a)

    # pair list: (chunk, tile, meta_col) grouped per window
    window_pairs = [[] for _ in range(NW)]
    mcol = 0
    for w in range(NW):
        for c in (0, 1):
            if R[w, c] == 0:
                continue
            t0 = pos[w, c] // 128
            t1 = (pos[w, c] + R[w, c] - 1) // 128
            for t in range(t0, t1 + 1):
                window_pairs[w].append((c, t, mcol))
                mcol += 1
    TP = mcol

    # per-core gather index streams + meta columns
    per_core = []
    for cc in range(NCORES):
        idx_streams = [np.zeros(L[c], np.int64) for c in (0, 1)]
        dstl_streams = [np.full(L[c], SENT, np.float32) for c in (0, 1)]
        for c in (0, 1):
            sel = order[(core[order] == cc) & (chunk[order] == c)]  # by window
            cnt = counts[cc, :, c]
            starts = pos[:, c]
            within = np.arange(sel.shape[0]) - np.repeat(
                np.concatenate([[0], np.cumsum(cnt[:-1])]), cnt
            )
            p = np.repeat(starts, cnt) + within
            rr = src[sel] % OWN
            tbl_row = (src[sel] // OWN) * HALFR + (rr - c * HALFR)
            idx_streams[c][p] = tbl_row
            dstl_streams[c][p] = (dst[sel] % OWN - win[sel] * WIN).astype(np.float32)
            assert tbl_row.max(initial=0) < TBL

        meta = np.full((128, TP), SENT, np.float32)
        for w in range(NW):
            for c, t, m in window_pairs[w]:
                seg_w = wof[c][t * 128:(t + 1) * 128]
                seg_d = dstl_streams[c][t * 128:(t + 1) * 128]
                meta[:, m] = np.where(seg_w == w, seg_d, SENT)

        imgs = []
        for c in (0, 1):
            a = idx_streams[c].astype(np.int16)
            img = a.reshape(-1, 16).T.copy()          # [16, L/16]
            img = np.tile(img, (8, 1))                # replicate across groups
            imgs.append(np.ascontiguousarray(img))
        per_core.append({"idxA": imgs[0], "idxB": imgs[1], "meta": meta})

    sched = {
        "window_pairs": window_pairs,
        "L": [int(L[0]), int(L[1])],
        "TP": TP,
    }
    return sched, per_core


# --------------------------------------------------------------------------
# device kernel builder
# --------------------------------------------------------------------------

def _build_nc(sched, has_bias):
    import concourse.bacc as bacc
    import concourse.mybir as mybir
    import concourse.tile as tile

    f32 = mybir.dt.float32
    i16 = mybir.dt.int16
    AF = mybir.ActivationFunctionType
    ALU = mybir.AluOpType
    X = mybir.AxisListType.X
    RG = [list(range(NCORES))]

    LA, LB = sched["L"]
    TP = sched["TP"]
    window_pairs = sched["window_pairs"]

    nc = bacc.Bacc("TRN2", target_bir_lowering=False, debug=False,
                   num_devices=NCORES)

    xT_t = nc.dram_tensor("xT_own", [F0, OWN_PAD], f32, kind="ExternalInput")
    idxA_t = nc.dram_tensor("idxA", [128, LA // 16], i16, kind="ExternalInput")
    idxB_t = nc.dram_tensor("idxB", [128, LB // 16], i16, kind="ExternalInput")
    meta_t = nc.dram_tensor("meta", [128, TP], f32, kind="ExternalInput")
    dinv_t = nc.dram_tensor("dinv_img", [128, NW], f32, kind="ExternalInput")
    ar_t = nc.dram_tensor("arange4", [128, 512], f32, kind="ExternalInput")
    id_t = nc.dram_tensor("ident", [128, 128], f32, kind="ExternalInput")
    w1_t = nc.dram_tensor("W1", [F0, F1], f32, kind="ExternalInput")
    w2_t = nc.dram_tensor("W2", [F1, F2], f32, kind="ExternalInput")
    w3_t = nc.dram_tensor("W3", [F2, F3], f32, kind="ExternalInput")
    if has_bias:
        b1_t = nc.dram_tensor("b1", [1, F1], f32, kind="ExternalInput")
        b2_t = nc.dram_tensor("b2", [1, F2], f32, kind="ExternalInput")
        b3_t = nc.dram_tensor("b3", [1, F3], f32, kind="ExternalInput")
        sqd_t = nc.dram_tensor("sqrtdeg", [1, OWN_PAD], f32, kind="ExternalInput")
    out_t = nc.dram_tensor("out", [OWN, F3], f32, kind="ExternalOutput")

    with tile.TileContext(nc) as tc:
        with (
            tc.tile_pool(name="dram", bufs=1, space="DRAM") as dram,
            tc.tile_pool(name="const", bufs=1) as cst,
            tc.tile_pool(name="sb", bufs=2) as sb,
            tc.tile_pool(name="mpool", bufs=3) as mp,
            tc.tile_pool(name="spool", bufs=6) as sp,
            tc.tile_pool(name="ps", bufs=2, space="PSUM") as ps,
            tc.tile_pool(name="ps1", bufs=1, space="PSUM") as ps1,
        ):
            # ---- resident constants ----
            ar_sb = cst.tile([128, 512], f32)
            nc.sync.dma_start(ar_sb[:], ar_t.ap())
            id_sb = cst.tile([128, 128], f32)
            nc.sync.dma_start(id_sb[:], id_t.ap())
            dinv_sb = cst.tile([128, NW], f32)
            nc.sync.dma_start(dinv_sb[:], dinv_t.ap())
            meta_sb = cst.tile([128, TP], f32)
            nc.sync.dma_start(meta_sb[:], meta_t.ap())
            idx_sb = []
            for name, t_, Lc in (("ia", idxA_t, LA), ("ib", idxB_t, LB)):
                tl = cst.tile([128, Lc // 16], i16, name=name)
                nc.sync.dma_start(tl[:], t_.ap())
                idx_sb.append(tl)
            w_sb = []
            for name, t_, fi, fo in (("w1", w1_t, F0, F1), ("w2", w2_t, F1, F2),
                                     ("w3", w3_t, F2, F3)):
                kt = fi // 128
                tl = cst.tile([128, kt * fo], f32, name=name)
                nc.sync.dma_start(
                    tl[:].rearrange("p (k f) -> p k f", k=kt),
                    t_.ap().rearrange("(k p) f -> p k f", p=128))
                w_sb.append(tl)
            b_sb = []
            sqd_sb = None
            if has_bias:
                for name, t_, fo in (("b1s", b1_t, F1), ("b2s", b2_t, F2),
                                     ("b3s", b3_t, F3)):
                    tl = cst.tile([1, fo], f32, name=name)
                    nc.sync.dma_start(tl[:], t_.ap())
                    b_sb.append(tl)
                sqd_sb = cst.tile([1, OWN_PAD], f32)
                nc.sync.dma_start(sqd_sb[:], sqd_t.ap())
            z_big = cst.tile([128, NW * F3], f32)

            # ---- DRAM intermediates ----
            agA = [dram.tile([HALFR, f], f32, name=f"agA{i}")
                   for i, f in enumerate((F1, F2, F3))]
            agB = [dram.tile([OWN_PAD - HALFR, f], f32, name=f"agB{i}")
                   for i, f in enumerate((F1, F2, F3))]
            tblA = [dram.tile([TBL, f], f32, name=f"tA{i}", addr_space="Shared")
                    for i, f in enumerate((F1, F2, F3))]
            tblB = [dram.tile([TBL, f], f32, name=f"tB{i}", addr_space="Shared")
                    for i, f in enumerate((F1, F2, F3))]
            BW = HALFR // 128           # full windows in the A chunk
            BCUT = HALFR - BW * 128     # rows of the boundary window in A

            def dinv_col(w):
                return dinv_sb[:, w:w + 1]

            def write_h(hb, w, li):
                a, b = agA[li], agB[li]
                if w < BW:
                    nc.sync.dma_start(a[w * 128:(w + 1) * 128, :], hb[:])
                elif w == BW and BCUT > 0:
                    nc.sync.dma_start(a[BW * 128:HALFR, :], hb[:BCUT, :])
                    nc.sync.dma_start(b[0:128 - BCUT, :], hb[BCUT:, :])
                else:
                    o = w * 128 - HALFR
                    nc.sync.dma_start(b[o:o + 128, :], hb[:])

            def read_own(ob, w, li):
                a, b = agA[li], agB[li]
                if w < BW:
                    nc.sync.dma_start(ob[:], a[w * 128:(w + 1) * 128, :])
                elif w == BW and BCUT > 0:
                    nc.sync.dma_start(ob[:BCUT, :], a[BW * 128:HALFR, :])
                    nc.sync.dma_start(ob[BCUT:, :], b[0:128 - BCUT, :])
                else:
                    o = w * 128 - HALFR
                    nc.sync.dma_start(ob[:], b[o:o + 128, :])

            def emit_agA(li):
                nc.gpsimd.collective_compute(
                    "AllGather", ALU.bypass, replica_groups=RG,
                    ins=[agA[li][:].opt()], outs=[tblA[li][:].opt()])

            def emit_agB(li):
                nc.gpsimd.collective_compute(
                    "AllGather", ALU.bypass, replica_groups=RG,
                    ins=[agB[li][0:OWN - HALFR, :].opt()],
                    outs=[tblB[li][:].opt()])

            # ---- GEMM for one 128-row block (node-major in and out) ----
            def gemm_block(blk, w, fi, fo, wsb, li_next):
                kt = fi // 128
                psg = ps.tile([128, fo], f32, name="psg", tag="psg")
                for k in range(kt):
                    pst = ps.tile([128, 128], f32, name="pst", tag="pst")
                    nc.tensor.transpose(pst[:], blk[:, k * 128:(k + 1) * 128],
                                        id_sb[:])
                    hT = sb.tile([128, 128], f32, name="hT", tag="hT", bufs=8)
                    nc.scalar.copy(hT[:], pst[:])
                    nc.tensor.matmul(psg[:], lhsT=hT[:],
                                     rhs=wsb[:, k * fo:(k + 1) * fo],
                                     start=(k == 0), stop=(k == kt - 1))
                hb = sb.tile([128, fo], f32, name="hb", tag="hb")
                nc.scalar.mul(hb[:], psg[:], dinv_col(w))
                write_h(hb, w, li_next)

            # ---- phase 0: layer-1 GEMM over own rows (x pre-transposed) ----
            for w in range(NW):
                kt = F0 // 128
                psg = ps.tile([128, F1], f32, name="psg", tag="psg")
                for k in range(kt):
                    hT = sb.tile([128, 128], f32, name="hT", tag="hT", bufs=8)
                    nc.sync.dma_start(
                        hT[:],
                        xT_t.ap()[k * 128:(k + 1) * 128,
                                  w * 128:(w + 1) * 128])
                    nc.tensor.matmul(psg[:], lhsT=hT[:],
                                     rhs=w_sb[0][:, k * F1:(k + 1) * F1],
                                     start=(k == 0), stop=(k == kt - 1))
                hb = sb.tile([128, F1], f32, name="hb", tag="hb")
                nc.scalar.mul(hb[:], psg[:], dinv_col(w))
                write_h(hb, w, 0)
                if w == (BW if BCUT > 0 else BW - 1):
                    emit_agA(0)
            emit_agB(0)

            # ---- layers ----
            for li, fo in enumerate((F1, F2, F3)):
                tviews = (tblA[li][:], tblB[li][:])

                live = [{}, {}]
                emitted = [0, 0]
                s4_live = {}
                s4_next = [0]
                Gl = GL[li]
                CALLl = Gl * 128

                def ensure(c, t, fo=fo, tviews=tviews, live=live,
                           emitted=emitted, Gl=Gl, CALLl=CALLl):
                    call = t // Gl
                    while emitted[c] <= call:
                        ci = emitted[c]
                        m = mp.tile([128, Gl * fo], f32, name=f"m{c}",
                                    tag=f"m{c}")
                        nc.gpsimd.dma_gather(
                            m[:].rearrange("p (t f) -> p t f", f=fo),
                            tviews[c],
                            idx_sb[c][:, ci * (CALLl // 16):(ci + 1) * (CALLl // 16)],
                            CALLl, CALLl, fo)
                        live[c][ci] = m
                        emitted[c] += 1
                    return live[c][call]

                def ensure_s4(mcol, s4_live=s4_live, s4_next=s4_next):
                    b = mcol // 4
                    while s4_next[0] <= b:
                        bi = s4_next[0]
                        m0 = bi * 4
                        nb = min(4, TP - m0)
                        s4 = sp.tile([128, 512], f32, name="s4", tag="s4")
                        nc.vector.tensor_tensor(
                            out=s4[:].rearrange("p (a f) -> p a f", a=4)[:, :nb, :],
                            in0=ar_sb[:].rearrange("p (a f) -> p a f", a=4)[:, :nb, :],
                            in1=meta_sb[:, m0:m0 + nb].to_broadcast([128, nb, 128]),
                            op=ALU.is_equal)
                        s4_live[bi] = s4
                        s4_next[0] += 1
                    return s4_live[b]

                ensure(0, 3 * Gl - 1)   # prefetch stream-A while AG-B runs

                for w in range(NW):
                    psw = ps.tile([128, fo], f32, name="psw", tag="psw")
                    first = True
                    for c, t, mcolv in window_pairs[w]:
                        m = ensure(c, t)
                        s4 = ensure_s4(mcolv)
                        j = mcolv % 4
                        sl = t % Gl
                        nc.tensor.matmul(psw[:],
                                         lhsT=s4[:, j * 128:(j + 1) * 128],
                                         rhs=m[:, sl * fo:(sl + 1) * fo],
                                         start=first, stop=False)
                        first = False
                    ob = sb.tile([128, fo], f32, name="ob", tag="ob")
                    read_own(ob, w, li)
                    stop_here = not has_bias
                    nc.tensor.matmul(psw[:], lhsT=id_sb[:], rhs=ob[:],
                                     start=first, stop=stop_here)
                    if has_bias:
                        nc.tensor.matmul(
                            psw[:],
                            lhsT=sqd_sb[0:1, w * 128:(w + 1) * 128],
                            rhs=b_sb[li][0:1, :], start=False, stop=True)
                    if li < 2:
                        hb = sb.tile([128, fo], f32, name="hbw", tag="hbw")
                        nc.scalar.activation(hb[:], psw[:], AF.Relu, bias=0.0,
                                             scale=dinv_col(w))
                        gemm_block(hb, w, fo, (F2, F3)[li], w_sb[li + 1],
                                   li + 1)
                        if w == (BW if BCUT > 0 else BW - 1):
                            emit_agA(li + 1)
                    else:
                        nc.scalar.activation(
                            z_big[:, w * F3:(w + 1) * F3], psw[:], AF.Relu,
                            bias=0.0, scale=dinv_col(w))
                if li < 2:
                    emit_agB(li + 1)

            # ---- finals ----
            zsum = cst.tile([128, 1], f32)
            nc.vector.reduce_sum(zsum[:], z_big[:], axis=X)
            ones = cst.tile([128, 1], f32)
            nc.vector.memset(ones[:], 1.0)
            pss = ps1.tile([128, 16], f32)
            nc.tensor.matmul(pss[0:1, 0:1], lhsT=ones[:], rhs=zsum[:],
                             start=True, stop=True)
            tot_sb = cst.tile([1, 16], f32)
            nc.vector.memset(tot_sb[:], 0.0)
            nc.scalar.copy(tot_sb[0:1, 0:1], pss[0:1, 0:1])
            ar_in = dram.tile([1, 16], f32)
            ar_out = dram.tile([1, 16], f32, addr_space="Shared")
            nc.sync.dma_start(ar_in[:], tot_sb[:])
            nc.gpsimd.collective_compute(
                "AllReduce", ALU.add, replica_groups=RG,
                ins=[ar_in[:].opt()], outs=[ar_out[:].opt()])
            tot2 = cst.tile([1, 16], f32)
            nc.sync.dma_start(tot2[:], ar_out[:])
            tot_bc = cst.tile([128, 1], f32)
            nc.gpsimd.partition_broadcast(tot_bc[:], tot2[0:1, 0:1],
                                          channels=128)
            inv_tot = cst.tile([128, 1], f32)
            nc.vector.reciprocal(inv_tot[:], tot_bc[:])

            # z/sum -> tanh -> ^2 ; then row L2 norm, all column-batched.
            # Ping-pong z_big <-> scr to bound SBUF: scr=tanh(z);
            # z_big=tanh^2; scr=tanh^4; reduce; scr=z_big*rinv; DMA out.
            scr = cst.tile([128, NW * F3], f32)
            nc.scalar.activation(scr[:], z_big[:], AF.Tanh, bias=0.0,
                                 scale=inv_tot[:])
            nc.scalar.square(z_big[:], scr[:])
            nc.scalar.square(scr[:], z_big[:])
            s4s = cst.tile([128, NW], f32)
            nc.vector.reduce_sum(
                s4s[:].rearrange("p w -> p w ()"),
                scr[:].rearrange("p (w f) -> p w f", w=NW), axis=X)
            nmr = cst.tile([128, NW], f32)
            nc.scalar.sqrt(nmr[:], s4s[:])
            rinv = cst.tile([128, NW], f32)
            nc.vector.reciprocal(rinv[:], nmr[:])
            nc.vector.tensor_scalar_min(rinv[:], rinv[:], 1.0 / EPS)
            nc.vector.tensor_tensor(
                out=scr[:].rearrange("p (w f) -> p w f", w=NW),
                in0=z_big[:].rearrange("p (w f) -> p w f", w=NW),
                in1=rinv[:].to_broadcast([128, NW, F3]),
                op=ALU.mult)
            # write out: full windows in one strided DMA, tail separate
            nc.sync.dma_start(
                out_t.ap()[0:(NW - 1) * 128, :].rearrange(
                    "(w p) f -> p w f", p=128),
                scr[:].rearrange("p (w f) -> p w f", w=NW)[:, 0:NW - 1, :])
            tail = OWN - (NW - 1) * 128
            nc.sync.dma_start(
                out_t.ap()[(NW - 1) * 128:OWN, :],
                scr[0:tail, (NW - 1) * F3:NW * F3])

    nc.compile()
    return nc


# --------------------------------------------------------------------------
# entry point
# --------------------------------------------------------------------------

def kernel(x, edge_index, W1, b1, W2, b2, W3, b3):
    from concourse.bass_utils import run_bass_kernel_spmd

    x = np.ascontiguousarray(np.asarray(x, dtype=np.float32))
    ei = np.asarray(edge_index)
    src = np.ascontiguousarray(ei[0]).astype(np.int64)
    dst = np.ascontiguousarray(ei[1]).astype(np.int64)
    W1 = np.ascontiguousarray(np.asarray(W1, np.float32))
    W2 = np.ascontiguousarray(np.asarray(W2, np.float32))
    W3 = np.ascontiguousarray(np.asarray(W3, np.float32))
    b1 = np.asarray(b1, np.float32)
    b2 = np.asarray(b2, np.float32)
    b3 = np.asarray(b3, np.float32)
    has_bias = bool(np.any(b1) or np.any(b2) or np.any(b3))

    deg = (np.bincount(dst, minlength=N) + 1.0).astype(np.float32)
    dinv = (1.0 / np.sqrt(deg.astype(np.float64))).astype(np.float32)

    ck = hash((src.tobytes(), dst.tobytes(), has_bias))
    if ck in _BUILD_CACHE:
        nc, sched, per_core = _BUILD_CACHE[ck]
    else:
        sched, per_core = _build_schedule(src, dst)
        nc = _build_nc(sched, has_bias)
        _BUILD_CACHE[ck] = (nc, sched, per_core)

    arange4 = np.tile(np.arange(128, dtype=np.float32)[None, :], (128, 4))
    ident = np.eye(128, dtype=np.float32)
    in_maps = []
    for c in range(NCORES):
        lo = c * OWN
        xT_own = np.zeros((F0, OWN_PAD), np.float32)
        xT_own[:, :OWN] = x[lo:lo + OWN].T
        dv = np.ones(OWN_PAD, np.float32)
        dv[:OWN] = dinv[lo:lo + OWN]
        dinv_img = np.ascontiguousarray(dv.reshape(NW, 128).T)
        m = {
            "xT_own": xT_own,
            "idxA": per_core[c]["idxA"],
            "idxB": per_core[c]["idxB"],
            "meta": per_core[c]["meta"],
            "dinv_img": dinv_img,
            "arange4": arange4,
            "ident": ident,
            "W1": W1, "W2": W2, "W3": W3,
        }
        if has_bias:
            sq = np.zeros((1, OWN_PAD), np.float32)
            sq[0, :OWN] = np.sqrt(deg[lo:lo + OWN])
            m["b1"] = b1.reshape(1, F1)
            m["b2"] = b2.reshape(1, F2)
            m["b3"] = b3.reshape(1, F3)
            m["sqrtdeg"] = sq
        in_maps.append(m)

    res = run_bass_kernel_spmd(nc, in_maps, core_ids=list(range(NCORES)),
                               **_RUN_KWARGS)
    global _LAST
    _LAST = res
    out = np.concatenate([res.results[c]["out"] for c in range(NCORES)], axis=0)
    return out


# test.py hooks (harness never touches these)
_RUN_KWARGS = {}
_LAST = None



# revision 3
# speedup vs baseline: 1.2896x; 1.2896x over previous
"""3-layer GCN (PyG GCNConv-style) Bass/Trainium2 kernel, 8-way SPMD.

Strategy (standard 1D graph partitioning, dst-sharded):
  - Core c owns node rows [c*6250, (c+1)*6250).
  - Per layer: local GEMM H = X@W (PE-transposed activations, W as moving
    operand), scaled by dinv -> Htilde; one AllGather -> full table [50000,F];
    gathers read it as two 25000-row views so indices fit int16.
  - Message passing: dst-sorted edges, bulk dma_gather of source rows,
    aggregated per 128-node window with one-hot "selection" matmuls
    (S[e, n] = 1 iff dst_local[e]==n) accumulating in PSUM; self-loop added
    via identity matmul; out = relu(dinv * psum). S matrices are built 4 per
    DVE op (batched is_equal against broadcast dst_local columns).
  - All tables / messages / weights are bf16 (PSUM accumulates fp32):
    4x PE matmul rate vs fp32, half the gather + AllGather bytes.
  - Schedule (runs per (window, src-half) padded to the max over cores) is
    identical on all cores -> single NEFF; per-core data lives in the
    gather-index / dst-local metadata input tensors.
  - Finals: z kept in SBUF fp32, global sum via AllReduce, z/sum -> tanh^2 ->
    row L2 normalize, all column-batched.
"""

import numpy as np
import ml_dtypes

BF16 = ml_dtypes.bfloat16

# ---- problem constants (hardcoded per contest contract) ----
N = 50000
F0, F1, F2, F3 = 512, 512, 256, 128
NCORES = 8
OWN = N // NCORES            # 6250 rows per core
WIN = 128
NW = (OWN + WIN - 1) // WIN  # 49 windows
OWN_PAD = NW * WIN           # 6272
HALFR = OWN // 2             # 3125: per-rank row split for the 2-chunk AllGather
TBL = NCORES * HALFR         # 25000 rows per gather table (< 32768, int16 ok)
GL = (8, 8, 8)               # gather tiles per dma_gather call, per layer
CALL = max(GL) * 128         # stream padding granularity (covers all layers)
SENT = 65000.0               # dst_local sentinel -> never matches iota 0..127
EPS = 1e-12

_BUILD_CACHE = {}


# --------------------------------------------------------------------------
# host-side schedule construction (pure index bookkeeping)
# --------------------------------------------------------------------------

def _build_schedule(src, dst):
    """Returns (sched, per_core) where sched is core-independent."""
    src = src.astype(np.int64)
    dst = dst.astype(np.int64)
    core = dst // OWN
    win = (dst % OWN) // WIN
    r = src % OWN
    chunk = (r >= HALFR).astype(np.int64)

    key = (core * NW + win) * 2 + chunk
    order = np.argsort(key, kind="stable")
    counts = np.bincount(key, minlength=NCORES * NW * 2).reshape(NCORES, NW, 2)
    R = counts.max(axis=0)                      # [NW, 2] padded run lengths
    pos = np.zeros((NW, 2), np.int64)           # start position of run (w,c)
    pos[1:, 0] = np.cumsum(R[:-1, 0])
    pos[1:, 1] = np.cumsum(R[:-1, 1])
    slen = R.sum(axis=0)                        # [2] stream lengths
    L = ((slen + CALL - 1) // CALL) * CALL      # padded to gather-call multiple

    # window-of-position per stream (pads extend each run; tail -> -1)
    wof = []
    for c in (0, 1):
        a = np.full(L[c], -1, np.int64)
        a[: slen[c]] = np.repeat(np.arange(NW), R[:, c])
        wof.append(a)

    # pair list: (chunk, tile, meta_col) grouped per window
    window_pairs = [[] for _ in range(NW)]
    mcol = 0
    for w in range(NW):
        for c in (0, 1):
            if R[w, c] == 0:
                continue
            t0 = pos[w, c] // 128
            t1 = (pos[w, c] + R[w, c] - 1) // 128
            for t in range(t0, t1 + 1):
                window_pairs[w].append((c, t, mcol))
                mcol += 1
    TP = mcol

    # per-core gather index streams + meta columns
    per_core = []
    for cc in range(NCORES):
        idx_streams = [np.zeros(L[c], np.int64) for c in (0, 1)]
        dstl_streams = [np.full(L[c], SENT, np.float32) for c in (0, 1)]
        for c in (0, 1):
            sel = order[(core[order] == cc) & (chunk[order] == c)]  # by window
            cnt = counts[cc, :, c]
            starts = pos[:, c]
            within = np.arange(sel.shape[0]) - np.repeat(
                np.concatenate([[0], np.cumsum(cnt[:-1])]), cnt
            )
            p = np.repeat(starts, cnt) + within
            rr = src[sel] % OWN
            tbl_row = (src[sel] // OWN) * HALFR + (rr - c * HALFR)
            idx_streams[c][p] = tbl_row
            dstl_streams[c][p] = (dst[sel] % OWN - win[sel] * WIN).astype(np.float32)
            assert tbl_row.max(initial=0) < TBL

        meta = np.full((128, TP), SENT, np.float32)
        for w in range(NW):
            for c, t, m in window_pairs[w]:
                seg_w = wof[c][t * 128:(t + 1) * 128]
                seg_d = dstl_streams[c][t * 128:(t + 1) * 128]
                meta[:, m] = np.where(seg_w == w, seg_d, SENT)

        imgs = []
        for c in (0, 1):
            a = idx_streams[c].astype(np.int16)
            img = a.reshape(-1, 16).T.copy()          # [16, L/16]
            img = np.tile(img, (8, 1))                # replicate across groups
            imgs.append(np.ascontiguousarray(img))
        per_core.append({"idxA": imgs[0], "idxB": imgs[1],
                         "meta": meta.astype(BF16)})

    sched = {
        "window_pairs": window_pairs,
        "L": [int(L[0]), int(L[1])],
        "TP": TP,
    }
    return sched, per_core


# --------------------------------------------------------------------------
# device kernel builder
# --------------------------------------------------------------------------

def _build_nc(sched, has_bias):
    import concourse.bacc as bacc
    import concourse.mybir as mybir
    import concourse.tile as tile

    f32 = mybir.dt.float32
    bf16 = mybir.dt.bfloat16
    i16 = mybir.dt.int16
    AF = mybir.ActivationFunctionType
    ALU = mybir.AluOpType
    X = mybir.AxisListType.X
    RG = [list(range(NCORES))]

    LA, LB = sched["L"]
    TP = sched["TP"]
    window_pairs = sched["window_pairs"]

    nc = bacc.Bacc("TRN2", target_bir_lowering=False, debug=False,
                   num_devices=NCORES)

    xT_t = nc.dram_tensor("xT_own", [F0, OWN_PAD], bf16, kind="ExternalInput")
    idxA_t = nc.dram_tensor("idxA", [128, LA // 16], i16, kind="ExternalInput")
    idxB_t = nc.dram_tensor("idxB", [128, LB // 16], i16, kind="ExternalInput")
    meta_t = nc.dram_tensor("meta", [128, TP], bf16, kind="ExternalInput")
    dinv_t = nc.dram_tensor("dinv_img", [128, NW], f32, kind="ExternalInput")
    ar_t = nc.dram_tensor("arange4", [128, 512], bf16, kind="ExternalInput")
    id_t = nc.dram_tensor("ident", [128, 128], bf16, kind="ExternalInput")
    w1_t = nc.dram_tensor("W1", [F0, F1], bf16, kind="ExternalInput")
    w2_t = nc.dram_tensor("W2", [F1, F2], bf16, kind="ExternalInput")
    w3_t = nc.dram_tensor("W3", [F2, F3], bf16, kind="ExternalInput")
    if has_bias:
        b1_t = nc.dram_tensor("b1", [1, F1], bf16, kind="ExternalInput")
        b2_t = nc.dram_tensor("b2", [1, F2], bf16, kind="ExternalInput")
        b3_t = nc.dram_tensor("b3", [1, F3], bf16, kind="ExternalInput")
        sqd_t = nc.dram_tensor("sqrtdeg", [1, OWN_PAD], bf16, kind="ExternalInput")
    out_t = nc.dram_tensor("out", [OWN, F3], f32, kind="ExternalOutput")

    with tile.TileContext(nc) as tc:
        with (
            tc.tile_pool(name="dram", bufs=1, space="DRAM") as dram,
            tc.tile_pool(name="const", bufs=1) as cst,
            tc.tile_pool(name="sb", bufs=2) as sb,
            tc.tile_pool(name="mpool", bufs=3) as mp,
            tc.tile_pool(name="spool", bufs=6) as sp,
            tc.tile_pool(name="ps", bufs=2, space="PSUM") as ps,
            tc.tile_pool(name="ps1", bufs=1, space="PSUM") as ps1,
        ):
            # ---- resident constants ----
            ar_sb = cst.tile([128, 512], bf16)
            nc.sync.dma_start(ar_sb[:], ar_t.ap())
            id_sb = cst.tile([128, 128], bf16)
            nc.sync.dma_start(id_sb[:], id_t.ap())
            dinv_sb = cst.tile([128, NW], f32)
            nc.sync.dma_start(dinv_sb[:], dinv_t.ap())
            meta_sb = cst.tile([128, TP], bf16)
            nc.sync.dma_start(meta_sb[:], meta_t.ap())
            idx_sb = []
            for name, t_, Lc in (("ia", idxA_t, LA), ("ib", idxB_t, LB)):
                tl = cst.tile([128, Lc // 16], i16, name=name)
                nc.sync.dma_start(tl[:], t_.ap())
                idx_sb.append(tl)
            w_sb = []
            for name, t_, fi, fo in (("w1", w1_t, F0, F1), ("w2", w2_t, F1, F2),
                                     ("w3", w3_t, F2, F3)):
                kt = fi // 128
                tl = cst.tile([128, kt * fo], bf16, name=name)
                nc.sync.dma_start(
                    tl[:].rearrange("p (k f) -> p k f", k=kt),
                    t_.ap().rearrange("(k p) f -> p k f", p=128))
                w_sb.append(tl)
            b_sb = []
            sqd_sb = None
            if has_bias:
                for name, t_, fo in (("b1s", b1_t, F1), ("b2s", b2_t, F2),
                                     ("b3s", b3_t, F3)):
                    tl = cst.tile([1, fo], bf16, name=name)
                    nc.sync.dma_start(tl[:], t_.ap())
                    b_sb.append(tl)
                sqd_sb = cst.tile([1, OWN_PAD], bf16)
                nc.sync.dma_start(sqd_sb[:], sqd_t.ap())
            z_big = cst.tile([128, NW * F3], f32)

            # ---- DRAM intermediates ----
            agA = [dram.tile([HALFR, f], bf16, name=f"agA{i}")
                   for i, f in enumerate((F1, F2, F3))]
            agB = [dram.tile([OWN_PAD - HALFR, f], bf16, name=f"agB{i}")
                   for i, f in enumerate((F1, F2, F3))]
            tblA = [dram.tile([TBL, f], bf16, name=f"tA{i}", addr_space="Shared")
                    for i, f in enumerate((F1, F2, F3))]
            tblB = [dram.tile([TBL, f], bf16, name=f"tB{i}", addr_space="Shared")
                    for i, f in enumerate((F1, F2, F3))]
            BW = HALFR // 128           # full windows in the A chunk
            BCUT = HALFR - BW * 128     # rows of the boundary window in A

            def dinv_col(w):
                return dinv_sb[:, w:w + 1]

            def write_h(hb, w, li):
                a, b = agA[li], agB[li]
                if w < BW:
                    nc.sync.dma_start(a[w * 128:(w + 1) * 128, :], hb[:])
                elif w == BW and BCUT > 0:
                    nc.sync.dma_start(a[BW * 128:HALFR, :], hb[:BCUT, :])
                    nc.sync.dma_start(b[0:128 - BCUT, :], hb[BCUT:, :])
                else:
                    o = w * 128 - HALFR
                    nc.sync.dma_start(b[o:o + 128, :], hb[:])

            def read_own(ob, w, li):
                a, b = agA[li], agB[li]
                if w < BW:
                    nc.sync.dma_start(ob[:], a[w * 128:(w + 1) * 128, :])
                elif w == BW and BCUT > 0:
                    nc.sync.dma_start(ob[:BCUT, :], a[BW * 128:HALFR, :])
                    nc.sync.dma_start(ob[BCUT:, :], b[0:128 - BCUT, :])
                else:
                    o = w * 128 - HALFR
                    nc.sync.dma_start(ob[:], b[o:o + 128, :])

            def emit_agA(li):
                nc.gpsimd.collective_compute(
                    "AllGather", ALU.bypass, replica_groups=RG,
                    ins=[agA[li][:].opt()], outs=[tblA[li][:].opt()])

            def emit_agB(li):
                nc.gpsimd.collective_compute(
                    "AllGather", ALU.bypass, replica_groups=RG,
                    ins=[agB[li][0:OWN - HALFR, :].opt()],
                    outs=[tblB[li][:].opt()])

            # ---- GEMM for one 128-row block (node-major in and out) ----
            def gemm_block(blk, w, fi, fo, wsb, li_next):
                kt = fi // 128
                psg = ps.tile([128, fo], f32, name="psg", tag="psg")
                for k in range(kt):
                    pst = ps.tile([128, 128], bf16, name="pst", tag="pst")
                    nc.tensor.transpose(pst[:], blk[:, k * 128:(k + 1) * 128],
                                        id_sb[:])
                    hT = sb.tile([128, 128], bf16, name="hT", tag="hT", bufs=8)
                    nc.scalar.copy(hT[:], pst[:])
                    nc.tensor.matmul(psg[:], lhsT=hT[:],
                                     rhs=wsb[:, k * fo:(k + 1) * fo],
                                     start=(k == 0), stop=(k == kt - 1))
                hb = sb.tile([128, fo], bf16, name="hb", tag="hb")
                nc.scalar.mul(hb[:], psg[:], dinv_col(w))
                write_h(hb, w, li_next)

            # ---- phase 0: layer-1 GEMM over own rows (x pre-transposed) ----
            for w in range(NW):
                kt = F0 // 128
                psg = ps.tile([128, F1], f32, name="psg", tag="psg")
                for k in range(kt):
                    hT = sb.tile([128, 128], bf16, name="hT", tag="hT", bufs=8)
                    nc.sync.dma_start(
                        hT[:],
                        xT_t.ap()[k * 128:(k + 1) * 128,
                                  w * 128:(w + 1) * 128])
                    nc.tensor.matmul(psg[:], lhsT=hT[:],
                                     rhs=w_sb[0][:, k * F1:(k + 1) * F1],
                                     start=(k == 0), stop=(k == kt - 1))
                hb = sb.tile([128, F1], bf16, name="hb", tag="hb")
                nc.scalar.mul(hb[:], psg[:], dinv_col(w))
                write_h(hb, w, 0)
                if w == (BW if BCUT > 0 else BW - 1):
                    emit_agA(0)
            emit_agB(0)

            # ---- layers ----
            for li, fo in enumerate((F1, F2, F3)):
                tviews = (tblA[li][:], tblB[li][:])

                live = [{}, {}]
                emitted = [0, 0]
                s4_live = {}
                s4_next = [0]
                Gl = GL[li]
                CALLl = Gl * 128

                def ensure(c, t, fo=fo, tviews=tviews, live=live,
                           emitted=emitted, Gl=Gl, CALLl=CALLl):
                    call = t // Gl
                    while emitted[c] <= call:
                        ci = emitted[c]
                        m = mp.tile([128, Gl * fo], bf16, name=f"m{c}",
                                    tag=f"m{c}")
                        nc.gpsimd.dma_gather(
                            m[:].rearrange("p (t f) -> p t f", f=fo),
                            tviews[c],
                            idx_sb[c][:, ci * (CALLl // 16):(ci + 1) * (CALLl // 16)],
                            CALLl, CALLl, fo)
                        live[c][ci] = m
                        emitted[c] += 1
                    return live[c][call]

                def ensure_s4(mcol, s4_live=s4_live, s4_next=s4_next):
                    b = mcol // 4
                    while s4_next[0] <= b:
                        bi = s4_next[0]
                        m0 = bi * 4
                        nb = min(4, TP - m0)
                        s4 = sp.tile([128, 512], bf16, name="s4", tag="s4")
                        nc.vector.tensor_tensor(
                            out=s4[:].rearrange("p (a f) -> p a f", a=4)[:, :nb, :],
                            in0=ar_sb[:].rearrange("p (a f) -> p a f", a=4)[:, :nb, :],
                            in1=meta_sb[:, m0:m0 + nb].to_broadcast([128, nb, 128]),
                            op=ALU.is_equal)
                        s4_live[bi] = s4
                        s4_next[0] += 1
                    return s4_live[b]

                ensure(0, 3 * Gl - 1)   # prefetch stream-A while AG-B runs

                for w in range(NW):
                    psw = ps.tile([128, fo], f32, name="psw", tag="psw")
                    first = True
                    for c, t, mcolv in window_pairs[w]:
                        m = ensure(c, t)
                        s4 = ensure_s4(mcolv)
                        j = mcolv % 4
                        sl = t % Gl
                        nc.tensor.matmul(psw[:],
                                         lhsT=s4[:, j * 128:(j + 1) * 128],
                                         rhs=m[:, sl * fo:(sl + 1) * fo],
                                         start=first, stop=False)
                        first = False
                    ob = sb.tile([128, fo], bf16, name="ob", tag="ob")
                    read_own(ob, w, li)
                    stop_here = not has_bias
                    nc.tensor.matmul(psw[:], lhsT=id_sb[:], rhs=ob[:],
                                     start=first, stop=stop_here)
                    if has_bias:
                        nc.tensor.matmul(
                            psw[:],
                            lhsT=sqd_sb[0:1, w * 128:(w + 1) * 128],
                            rhs=b_sb[li][0:1, :], start=False, stop=True)
                    if li < 2:
                        hb = sb.tile([128, fo], bf16, name="hbw", tag="hbw")
                        nc.scalar.activation(hb[:], psw[:], AF.Relu, bias=0.0,
                                             scale=dinv_col(w))
                        gemm_block(hb, w, fo, (F2, F3)[li], w_sb[li + 1],
                                   li + 1)
                        if w == (BW if BCUT > 0 else BW - 1):
                            emit_agA(li + 1)
                    else:
                        nc.scalar.activation(
                            z_big[:, w * F3:(w + 1) * F3], psw[:], AF.Relu,
                            bias=0.0, scale=dinv_col(w))
                if li < 2:
                    emit_agB(li + 1)

            # ---- finals ----
            zsum = cst.tile([128, 1], f32)
            nc.vector.reduce_sum(zsum[:], z_big[:], axis=X)
            ones = cst.tile([128, 1], f32)
            nc.vector.memset(ones[:], 1.0)
            pss = ps1.tile([128, 16], f32)
            nc.tensor.matmul(pss[0:1, 0:1], lhsT=ones[:], rhs=zsum[:],
                             start=True, stop=True)
            tot_sb = cst.tile([1, 16], f32)
            nc.vector.memset(tot_sb[:], 0.0)
            nc.scalar.copy(tot_sb[0:1, 0:1], pss[0:1, 0:1])
            ar_in = dram.tile([1, 16], f32)
            ar_out = dram.tile([1, 16], f32, addr_space="Shared")
            nc.sync.dma_start(ar_in[:], tot_sb[:])
            nc.gpsimd.collective_compute(
                "AllReduce", ALU.add, replica_groups=RG,
                ins=[ar_in[:].opt()], outs=[ar_out[:].opt()])
            tot2 = cst.tile([1, 16], f32)
            nc.sync.dma_start(tot2[:], ar_out[:])
            tot_bc = cst.tile([128, 1], f32)
            nc.gpsimd.partition_broadcast(tot_bc[:], tot2[0:1, 0:1],
                                          channels=128)
            inv_tot = cst.tile([128, 1], f32)
            nc.vector.reciprocal(inv_tot[:], tot_bc[:])

            # z/sum -> tanh -> ^2 ; then row L2 norm, all column-batched.
            # Ping-pong z_big <-> scr to bound SBUF: scr=tanh(z);
            # z_big=tanh^2; scr=tanh^4; reduce; scr=z_big*rinv; DMA out.
            scr = cst.tile([128, NW * F3], f32)
            nc.scalar.activation(scr[:], z_big[:], AF.Tanh, bias=0.0,
                                 scale=inv_tot[:])
            nc.scalar.square(z_big[:], scr[:])
            nc.scalar.square(scr[:], z_big[:])
            s4s = cst.tile([128, NW], f32)
            nc.vector.reduce_sum(
                s4s[:].rearrange("p w -> p w ()"),
                scr[:].rearrange("p (w f) -> p w f", w=NW), axis=X)
            nmr = cst.tile([128, NW], f32)
            nc.scalar.sqrt(nmr[:], s4s[:])
            rinv = cst.tile([128, NW], f32)
            nc.vector.reciprocal(rinv[:], nmr[:])
            nc.vector.tensor_scalar_min(rinv[:], rinv[:], 1.0 / EPS)
            nc.vector.tensor_tensor(
                out=scr[:].rearrange("p (w f) -> p w f", w=NW),
                in0=z_big[:].rearrange("p (w f) -> p w f", w=NW),
                in1=rinv[:].to_broadcast([128, NW, F3]),
                op=ALU.mult)
            # write out: full windows in one strided DMA, tail separate
            nc.sync.dma_start(
                out_t.ap()[0:(NW - 1) * 128, :].rearrange(
                    "(w p) f -> p w f", p=128),
                scr[:].rearrange("p (w f) -> p w f", w=NW)[:, 0:NW - 1, :])
            tail = OWN - (NW - 1) * 128
            nc.sync.dma_start(
                out_t.ap()[(NW - 1) * 128:OWN, :],
                scr[0:tail, (NW - 1) * F3:NW * F3])

    nc.compile()
    return nc


# --------------------------------------------------------------------------
# entry point
# --------------------------------------------------------------------------

def kernel(x, edge_index, W1, b1, W2, b2, W3, b3):
    from concourse.bass_utils import run_bass_kernel_spmd

    x = np.ascontiguousarray(np.asarray(x, dtype=np.float32))
    ei = np.asarray(edge_index)
    src = np.ascontiguousarray(ei[0]).astype(np.int64)
    dst = np.ascontiguousarray(ei[1]).astype(np.int64)
    W1 = np.ascontiguousarray(np.asarray(W1, np.float32)).astype(BF16)
    W2 = np.ascontiguousarray(np.asarray(W2, np.float32)).astype(BF16)
    W3 = np.ascontiguousarray(np.asarray(W3, np.float32)).astype(BF16)
    b1 = np.asarray(b1, np.float32)
    b2 = np.asarray(b2, np.float32)
    b3 = np.asarray(b3, np.float32)
    has_bias = bool(np.any(b1) or np.any(b2) or np.any(b3))

    deg = (np.bincount(dst, minlength=N) + 1.0).astype(np.float32)
    dinv = (1.0 / np.sqrt(deg.astype(np.float64))).astype(np.float32)

    ck = hash((src.tobytes(), dst.tobytes(), has_bias))
    if ck in _BUILD_CACHE:
        nc, sched, per_core = _BUILD_CACHE[ck]
    else:
        sched, per_core = _build_schedule(src, dst)
        nc = _build_nc(sched, has_bias)
        _BUILD_CACHE[ck] = (nc, sched, per_core)

    arange4 = np.tile(np.arange(128, dtype=np.float32)[None, :],
                      (128, 4)).astype(BF16)
    ident = np.eye(128, dtype=np.float32).astype(BF16)
    in_maps = []
    for c in range(NCORES):
        lo = c * OWN
        xT_own = np.zeros((F0, OWN_PAD), BF16)
        xT_own[:, :OWN] = x[lo:lo + OWN].T.astype(BF16)
        dv = np.ones(OWN_PAD, np.float32)
        dv[:OWN] = dinv[lo:lo + OWN]
        dinv_img = np.ascontiguousarray(dv.reshape(NW, 128).T)
        m = {
            "xT_own": xT_own,
            "idxA": per_core[c]["idxA"],
            "idxB": per_core[c]["idxB"],
            "meta": per_core[c]["meta"],
            "dinv_img": dinv_img,
            "arange4": arange4,
            "ident": ident,
            "W1": W1, "W2": W2, "W3": W3,
        }
        if has_bias:
            sq = np.zeros((1, OWN_PAD), np.float32)
            sq[0, :OWN] = np.sqrt(deg[lo:lo + OWN])
            m["b1"] = b1.reshape(1, F1).astype(BF16)
            m["b2"] = b2.reshape(1, F2).astype(BF16)
            m["b3"] = b3.reshape(1, F3).astype(BF16)
            m["sqrtdeg"] = sq.astype(BF16)
        in_maps.append(m)

    res = run_bass_kernel_spmd(nc, in_maps, core_ids=list(range(NCORES)),
                               **_RUN_KWARGS)
    global _LAST
    _LAST = res
    out = np.concatenate([res.results[c]["out"] for c in range(NCORES)], axis=0)
    return out


# test.py hooks (harness never touches these)
_RUN_KWARGS = {}
_LAST = None


# revision 18
# speedup vs baseline: 1.7555x; 1.3613x over previous
"""3-layer GCN (PyG GCNConv-style) Bass/Trainium2 kernel, 8-way SPMD.

Strategy (standard 1D graph partitioning, dst-sharded):
  - Core c owns node rows [c*6250, (c+1)*6250).
  - Per layer: local GEMM H = X@W (PE-transposed activations, W as moving
    operand), scaled by dinv -> Htilde; one AllGather -> full table [50000,F];
    gathers read it as two 25000-row views so indices fit int16.
  - Message passing: dst-sorted edges, bulk dma_gather of source rows,
    aggregated per 128-node window with one-hot "selection" matmuls
    (S[e, n] = 1 iff dst_local[e]==n) accumulating in PSUM; self-loop added
    via identity matmul; out = relu(dinv * psum). S matrices are built 4 per
    DVE op (batched is_equal against broadcast dst_local columns).
  - All tables / messages / weights are bf16 (PSUM accumulates fp32):
    4x PE matmul rate vs fp32, half the gather + AllGather bytes.
  - Schedule (runs per (window, src-half) padded to the max over cores) is
    identical on all cores -> single NEFF; per-core data lives in the
    gather-index / dst-local metadata input tensors.
  - Finals: z kept in SBUF fp32, global sum via AllReduce, z/sum -> tanh^2 ->
    row L2 normalize, all column-batched.
"""

import numpy as np
import ml_dtypes

BF16 = ml_dtypes.bfloat16

# ---- problem constants (hardcoded per contest contract) ----
N = 50000
F0, F1, F2, F3 = 512, 512, 256, 128
NCORES = 8
OWN = N // NCORES            # 6250 rows per core
WIN = 128
NW = (OWN + WIN - 1) // WIN  # 49 windows
OWN_PAD = NW * WIN           # 6272
HALFR = OWN // 2             # 3125: per-rank row split for the 2-chunk AllGather
TBL = NCORES * HALFR         # 25000 rows per gather table (< 32768, int16 ok)
GL = (8, 8, 8)               # gather tiles per dma_gather call, per layer
GATHER_MODE = "queues"         # "plain" | "queues" | "prep" (debug isolation)
CALL = max(GL) * 128         # stream padding granularity (covers all layers)
SENT = 65000.0               # dst_local sentinel -> never matches iota 0..127
EPS = 1e-12

_BUILD_CACHE = {}


# --------------------------------------------------------------------------
# host-side schedule construction (pure index bookkeeping)
# --------------------------------------------------------------------------

def _build_schedule(src, dst):
    """Returns (sched, per_core) where sched is core-independent."""
    src = src.astype(np.int64)
    dst = dst.astype(np.int64)
    core = dst // OWN
    win = (dst % OWN) // WIN
    r = src % OWN
    chunk = (r >= HALFR).astype(np.int64)

    key = (core * NW + win) * 2 + chunk
    order = np.argsort(key, kind="stable")
    counts = np.bincount(key, minlength=NCORES * NW * 2).reshape(NCORES, NW, 2)
    R = counts.max(axis=0)                      # [NW, 2] padded run lengths
    pos = np.zeros((NW, 2), np.int64)           # start position of run (w,c)
    pos[1:, 0] = np.cumsum(R[:-1, 0])
    pos[1:, 1] = np.cumsum(R[:-1, 1])
    slen = R.sum(axis=0)                        # [2] stream lengths
    L = ((slen + CALL - 1) // CALL) * CALL      # padded to gather-call multiple

    # window-of-position per stream (pads extend each run; tail -> -1)
    wof = []
    for c in (0, 1):
        a = np.full(L[c], -1, np.int64)
        a[: slen[c]] = np.repeat(np.arange(NW), R[:, c])
        wof.append(a)

    # pair list: (chunk, tile, meta_col) grouped per window
    window_pairs = [[] for _ in range(NW)]
    mcol = 0
    for w in range(NW):
        for c in (0, 1):
            if R[w, c] == 0:
                continue
            t0 = pos[w, c] // 128
            t1 = (pos[w, c] + R[w, c] - 1) // 128
            for t in range(t0, t1 + 1):
                window_pairs[w].append((c, t, mcol))
                mcol += 1
    TP = mcol

    # per-core gather index streams + meta columns
    per_core = []
    for cc in range(NCORES):
        idx_streams = [np.zeros(L[c], np.int64) for c in (0, 1)]
        dstl_streams = [np.full(L[c], SENT, np.float32) for c in (0, 1)]
        for c in (0, 1):
            sel = order[(core[order] == cc) & (chunk[order] == c)]  # by window
            cnt = counts[cc, :, c]
            starts = pos[:, c]
            within = np.arange(sel.shape[0]) - np.repeat(
                np.concatenate([[0], np.cumsum(cnt[:-1])]), cnt
            )
            p = np.repeat(starts, cnt) + within
            rr = src[sel] % OWN
            tbl_row = (src[sel] // OWN) * HALFR + (rr - c * HALFR)
            idx_streams[c][p] = tbl_row
            dstl_streams[c][p] = (dst[sel] % OWN - win[sel] * WIN).astype(np.float32)
            assert tbl_row.max(initial=0) < TBL

        meta = np.full((128, TP), SENT, np.float32)
        for w in range(NW):
            for c, t, m in window_pairs[w]:
                seg_w = wof[c][t * 128:(t + 1) * 128]
                seg_d = dstl_streams[c][t * 128:(t + 1) * 128]
                meta[:, m] = np.where(seg_w == w, seg_d, SENT)

        imgs = []
        for c in (0, 1):
            a = idx_streams[c].astype(np.int16)
            img = a.reshape(-1, 16).T.copy()          # [16, L/16]
            img = np.tile(img, (8, 1))                # replicate across groups
            imgs.append(np.ascontiguousarray(img))
        per_core.append({"idxA": imgs[0], "idxB": imgs[1],
                         "meta": meta.astype(BF16)})

    sched = {
        "window_pairs": window_pairs,
        "L": [int(L[0]), int(L[1])],
        "TP": TP,
    }
    return sched, per_core


# --------------------------------------------------------------------------
# device kernel builder
# --------------------------------------------------------------------------

def _build_nc(sched, has_bias):
    import concourse.bacc as bacc
    import concourse.mybir as mybir
    import concourse.tile as tile

    f32 = mybir.dt.float32
    bf16 = mybir.dt.bfloat16
    i16 = mybir.dt.int16
    AF = mybir.ActivationFunctionType
    ALU = mybir.AluOpType
    X = mybir.AxisListType.X
    RG = [list(range(NCORES))]

    LA, LB = sched["L"]
    TP = sched["TP"]
    window_pairs = sched["window_pairs"]

    nc = bacc.Bacc("TRN2", target_bir_lowering=False, debug=False,
                   num_devices=NCORES, num_swdge_queues=2)

    xT_t = nc.dram_tensor("xT_own", [F0, OWN_PAD], bf16, kind="ExternalInput")
    idxA_t = nc.dram_tensor("idxA", [128, LA // 16], i16, kind="ExternalInput")
    idxB_t = nc.dram_tensor("idxB", [128, LB // 16], i16, kind="ExternalInput")
    meta_t = nc.dram_tensor("meta", [128, TP], bf16, kind="ExternalInput")
    dinv_t = nc.dram_tensor("dinv_img", [128, NW], f32, kind="ExternalInput")
    ar_t = nc.dram_tensor("arange4", [128, 512], bf16, kind="ExternalInput")
    id_t = nc.dram_tensor("ident", [128, 128], bf16, kind="ExternalInput")
    w1_t = nc.dram_tensor("W1", [F0, F1], bf16, kind="ExternalInput")
    w2_t = nc.dram_tensor("W2", [F1, F2], bf16, kind="ExternalInput")
    w3_t = nc.dram_tensor("W3", [F2, F3], bf16, kind="ExternalInput")
    if has_bias:
        b1_t = nc.dram_tensor("b1", [1, F1], bf16, kind="ExternalInput")
        b2_t = nc.dram_tensor("b2", [1, F2], bf16, kind="ExternalInput")
        b3_t = nc.dram_tensor("b3", [1, F3], bf16, kind="ExternalInput")
        sqd_t = nc.dram_tensor("sqrtdeg", [1, OWN_PAD], bf16, kind="ExternalInput")
    out_t = nc.dram_tensor("out", [OWN, F3], f32, kind="ExternalOutput")

    with tile.TileContext(nc) as tc:
        with (
            tc.tile_pool(name="dram", bufs=1, space="DRAM") as dram,
            tc.tile_pool(name="const", bufs=1) as cst,
            tc.tile_pool(name="sb", bufs=2) as sb,
            tc.tile_pool(name="spool", bufs=6) as sp,
            tc.tile_pool(name="ps", bufs=2, space="PSUM") as ps,
            tc.tile_pool(name="ps1", bufs=1, space="PSUM") as ps1,
        ):
            # ---- resident constants ----
            ar_sb = cst.tile([128, 512], bf16)
            nc.sync.dma_start(ar_sb[:], ar_t.ap())
            id_sb = cst.tile([128, 128], bf16)
            nc.sync.dma_start(id_sb[:], id_t.ap())
            dinv_sb = cst.tile([128, NW], f32)
            nc.sync.dma_start(dinv_sb[:], dinv_t.ap())
            meta_sb = cst.tile([128, TP], bf16)
            nc.sync.dma_start(meta_sb[:], meta_t.ap())
            idx_sb = []
            for name, t_, Lc in (("ia", idxA_t, LA), ("ib", idxB_t, LB)):
                tl = cst.tile([128, Lc // 16], i16, name=name)
                nc.sync.dma_start(tl[:], t_.ap())
                idx_sb.append(tl)
            w_sb = []
            for name, t_, fi, fo in (("w1", w1_t, F0, F1), ("w2", w2_t, F1, F2),
                                     ("w3", w3_t, F2, F3)):
                kt = fi // 128
                tl = cst.tile([128, kt * fo], bf16, name=name)
                nc.sync.dma_start(
                    tl[:].rearrange("p (k f) -> p k f", k=kt),
                    t_.ap().rearrange("(k p) f -> p k f", p=128))
                w_sb.append(tl)
            b_sb = []
            sqd_sb = None
            if has_bias:
                for name, t_, fo in (("b1s", b1_t, F1), ("b2s", b2_t, F2),
                                     ("b3s", b3_t, F3)):
                    tl = cst.tile([1, fo], bf16, name=name)
                    nc.sync.dma_start(tl[:], t_.ap())
                    b_sb.append(tl)
                sqd_sb = cst.tile([1, OWN_PAD], bf16)
                nc.sync.dma_start(sqd_sb[:], sqd_t.ap())
            z_big = cst.tile([128, NW * F3], f32)

            # ---- DRAM intermediates ----
            agA = [dram.tile([HALFR, f], bf16, name=f"agA{i}")
                   for i, f in enumerate((F1, F2, F3))]
            agB = [dram.tile([OWN_PAD - HALFR, f], bf16, name=f"agB{i}")
                   for i, f in enumerate((F1, F2, F3))]
            tblA = [dram.tile([TBL, f], bf16, name=f"tA{i}", addr_space="Shared")
                    for i, f in enumerate((F1, F2, F3))]
            tblB = [dram.tile([TBL, f], bf16, name=f"tB{i}", addr_space="Shared")
                    for i, f in enumerate((F1, F2, F3))]
            BW = HALFR // 128           # full windows in the A chunk
            BCUT = HALFR - BW * 128     # rows of the boundary window in A

            def dinv_col(w):
                return dinv_sb[:, w:w + 1]

            def write_h(hb, w, li):
                a, b = agA[li], agB[li]
                if w < BW:
                    nc.sync.dma_start(a[w * 128:(w + 1) * 128, :], hb[:])
                elif w == BW and BCUT > 0:
                    nc.sync.dma_start(a[BW * 128:HALFR, :], hb[:BCUT, :])
                    nc.sync.dma_start(b[0:128 - BCUT, :], hb[BCUT:, :])
                else:
                    o = w * 128 - HALFR
                    nc.sync.dma_start(b[o:o + 128, :], hb[:])

            def read_own(ob, w, li):
                a, b = agA[li], agB[li]
                if w < BW:
                    nc.sync.dma_start(ob[:], a[w * 128:(w + 1) * 128, :])
                elif w == BW and BCUT > 0:
                    nc.sync.dma_start(ob[:BCUT, :], a[BW * 128:HALFR, :])
                    nc.sync.dma_start(ob[BCUT:, :], b[0:128 - BCUT, :])
                else:
                    o = w * 128 - HALFR
                    nc.sync.dma_start(ob[:], b[o:o + 128, :])

            def emit_agA(li):
                nc.gpsimd.collective_compute(
                    "AllGather", ALU.bypass, replica_groups=RG,
                    ins=[agA[li][:].opt()], outs=[tblA[li][:].opt()])

            def emit_agB(li):
                nc.gpsimd.collective_compute(
                    "AllGather", ALU.bypass, replica_groups=RG,
                    ins=[agB[li][0:OWN - HALFR, :].opt()],
                    outs=[tblB[li][:].opt()])

            # ---- GEMM for one 128-row block (node-major in and out) ----
            def gemm_block(blk, w, fi, fo, wsb, li_next):
                kt = fi // 128
                psg = ps.tile([128, fo], f32, name="psg", tag="psg")
                for k in range(kt):
                    pst = ps.tile([128, 128], bf16, name="pst", tag="pst")
                    nc.tensor.transpose(pst[:], blk[:, k * 128:(k + 1) * 128],
                                        id_sb[:])
                    hT = sb.tile([128, 128], bf16, name="hT", tag="hT", bufs=8)
                    nc.scalar.copy(hT[:], pst[:])
                    nc.tensor.matmul(psg[:], lhsT=hT[:],
                                     rhs=wsb[:, k * fo:(k + 1) * fo],
                                     start=(k == 0), stop=(k == kt - 1))
                hb = sb.tile([128, fo], bf16, name="hb", tag="hb")
                nc.scalar.mul(hb[:], psg[:], dinv_col(w))
                write_h(hb, w, li_next)

            # ---- phase 0: layer-1 GEMM over own rows (x pre-transposed) ----
            # x slabs loaded whole (4 big DMAs) into a scoped pool released
            # before the gather-message pool opens.
            kt0 = F0 // 128
            with tc.tile_pool(name="xsl", bufs=1) as xp:
                xsl = []
                for k in range(kt0):
                    t = xp.tile([128, OWN_PAD], bf16, name=f"xs{k}")
                    nc.sync.dma_start(t[:], xT_t.ap()[k * 128:(k + 1) * 128, :])
                    xsl.append(t)
                for w in range(NW):
                    psg = ps.tile([128, F1], f32, name="psg", tag="psg")
                    for k in range(kt0):
                        nc.tensor.matmul(psg[:],
                                         lhsT=xsl[k][:, w * 128:(w + 1) * 128],
                                         rhs=w_sb[0][:, k * F1:(k + 1) * F1],
                                         start=(k == 0), stop=(k == kt0 - 1))
                    hb = sb.tile([128, F1], bf16, name="hb", tag="hb")
                    nc.scalar.mul(hb[:], psg[:], dinv_col(w))
                    write_h(hb, w, 0)
                    if w == (BW if BCUT > 0 else BW - 1):
                        emit_agA(0)
                emit_agB(0)

            # ---- layers ----
            # Gathers are PREPARE_ONLY on two SWDGE queues (chunk A -> q0,
            # chunk B -> q1): descriptor generation (the serial Q7 cost) never
            # waits on an AllGather; the cheap trigger_dma carries the
            # table-ready dependency instead. Each prep must bake in the
            # DMASW-lane global sem Tile's wait pass will reference —
            # lanes round-robin over every SWDGE DMA in engine order.
            lane_sems = tc.sems.swdge_block()
            prep_count = [0]
            LOOK = 4
            with tc.tile_pool(name="mpool", bufs=6) as mp:
                for li, fo in enumerate((F1, F2, F3)):
                    tviews = (tblA[li][:], tblB[li][:])

                    live = [{}, {}]
                    prepped = [0, 0]
                    triggered = [0, 0]
                    s4_live = {}
                    s4_next = [0]
                    Gl = GL[li]
                    CALLl = Gl * 128
                    ncalls = [LA // CALLl, LB // CALLl]

                    def prep(c, hi, fo=fo, tviews=tviews, live=live,
                             prepped=prepped, Gl=Gl, CALLl=CALLl,
                             ncalls=ncalls):
                        hi = min(hi, ncalls[c] - 1)
                        while prepped[c] <= hi:
                            ci = prepped[c]
                            m = mp.tile([128, Gl * fo], bf16, name=f"m{c}",
                                        tag=f"m{c}")
                            kw = {}
                            if GATHER_MODE == "prep":
                                kw = dict(prepare_only=True,
                                          sem=lane_sems[prep_count[0]
                                                        % len(lane_sems)])
                                prep_count[0] += 1
                            elif GATHER_MODE == "queues":
                                kw = dict(queue_num=c)
                            nc.gpsimd.dma_gather(
                                m[:].rearrange("p (t f) -> p t f", f=fo),
                                tviews[c],
                                idx_sb[c][:, ci * (CALLl // 16):
                                          (ci + 1) * (CALLl // 16)],
                                CALLl, CALLl, fo, **kw)
                            live[c][ci] = m
                            prepped[c] += 1

                    def ensure(c, t, live=live, prepped=prepped,
                               triggered=triggered, Gl=Gl, ncalls=ncalls,
                               prep=prep):
                        call = t // Gl
                        if triggered[c] <= call:
                            prep(c, call + LOOK)
                            if GATHER_MODE == "prep":
                                # one shared queue: a trigger drains BOTH
                                # streams' pending preps
                                if (prepped[0] > triggered[0]
                                        or prepped[1] > triggered[1]):
                                    nc.gpsimd.trigger_dma(count=None)
                                triggered[0] = prepped[0]
                                triggered[1] = prepped[1]
                            else:
                                triggered[c] = prepped[c]
                        return live[c][call]

                    def ensure_s4(mcol, s4_live=s4_live, s4_next=s4_next):
                        b = mcol // 4
                        while s4_next[0] <= b:
                            bi = s4_next[0]
                            m0 = bi * 4
                            nb = min(4, TP - m0)
                            s4 = sp.tile([128, 512], bf16, name="s4", tag="s4")
                            nc.vector.tensor_tensor(
                                out=s4[:].rearrange(
                                    "p (a f) -> p a f", a=4)[:, :nb, :],
                                in0=ar_sb[:].rearrange(
                                    "p (a f) -> p a f", a=4)[:, :nb, :],
                                in1=meta_sb[:, m0:m0 + nb].to_broadcast(
                                    [128, nb, 128]),
                                op=ALU.is_equal)
                            s4_live[bi] = s4
                            s4_next[0] += 1
                        return s4_live[b]

                    # desc-gen warm-up: queue LOOK+1 calls deep on both
                    # streams before the first window consumes anything.
                    prep(0, LOOK)
                    prep(1, LOOK)

                    for w in range(NW):
                        psw = ps.tile([128, fo], f32, name="psw", tag="psw")
                        first = True
                        for c, t, mcolv in window_pairs[w]:
                            m = ensure(c, t)
                            s4 = ensure_s4(mcolv)
                            j = mcolv % 4
                            sl = t % Gl
                            nc.tensor.matmul(psw[:],
                                             lhsT=s4[:, j * 128:(j + 1) * 128],
                                             rhs=m[:, sl * fo:(sl + 1) * fo],
                                             start=first, stop=False)
                            first = False
                        ob = sb.tile([128, fo], bf16, name="ob", tag="ob")
                        read_own(ob, w, li)
                        stop_here = not has_bias
                        nc.tensor.matmul(psw[:], lhsT=id_sb[:], rhs=ob[:],
                                         start=first, stop=stop_here)
                        if has_bias:
                            nc.tensor.matmul(
                                psw[:],
                                lhsT=sqd_sb[0:1, w * 128:(w + 1) * 128],
                                rhs=b_sb[li][0:1, :], start=False, stop=True)
                        if li < 2:
                            hb = sb.tile([128, fo], bf16, name="hbw",
                                         tag="hbw")
                            nc.scalar.activation(hb[:], psw[:], AF.Relu,
                                                 bias=0.0, scale=dinv_col(w))
                            gemm_block(hb, w, fo, (F2, F3)[li], w_sb[li + 1],
                                       li + 1)
                            if w == (BW if BCUT > 0 else BW - 1):
                                emit_agA(li + 1)
                        else:
                            nc.scalar.activation(
                                z_big[:, w * F3:(w + 1) * F3], psw[:],
                                AF.Relu, bias=0.0, scale=dinv_col(w))
                    if li < 2:
                        emit_agB(li + 1)

            # ---- finals ----
            zsum = cst.tile([128, 1], f32)
            nc.vector.reduce_sum(zsum[:], z_big[:], axis=X)
            ones = cst.tile([128, 1], f32)
            nc.vector.memset(ones[:], 1.0)
            pss = ps1.tile([128, 16], f32)
            nc.tensor.matmul(pss[0:1, 0:1], lhsT=ones[:], rhs=zsum[:],
                             start=True, stop=True)
            tot_sb = cst.tile([1, 16], f32)
            nc.vector.memset(tot_sb[:], 0.0)
            nc.scalar.copy(tot_sb[0:1, 0:1], pss[0:1, 0:1])
            ar_in = dram.tile([1, 16], f32)
            ar_out = dram.tile([1, 16], f32, addr_space="Shared")
            nc.sync.dma_start(ar_in[:], tot_sb[:])
            nc.gpsimd.collective_compute(
                "AllReduce", ALU.add, replica_groups=RG,
                ins=[ar_in[:].opt()], outs=[ar_out[:].opt()])
            tot2 = cst.tile([1, 16], f32)
            nc.sync.dma_start(tot2[:], ar_out[:])
            tot_bc = cst.tile([128, 1], f32)
            nc.gpsimd.partition_broadcast(tot_bc[:], tot2[0:1, 0:1],
                                          channels=128)
            inv_tot = cst.tile([128, 1], f32)
            nc.vector.reciprocal(inv_tot[:], tot_bc[:])

            # z/sum -> tanh -> ^2 ; then row L2 norm, all column-batched.
            # Ping-pong z_big <-> scr to bound SBUF: scr=tanh(z);
            # z_big=tanh^2; scr=tanh^4; reduce; scr=z_big*rinv; DMA out.
            scr = cst.tile([128, NW * F3], f32)
            nc.scalar.activation(scr[:], z_big[:], AF.Tanh, bias=0.0,
                                 scale=inv_tot[:])
            nc.scalar.square(z_big[:], scr[:])
            nc.scalar.square(scr[:], z_big[:])
            s4s = cst.tile([128, NW], f32)
            nc.vector.reduce_sum(
                s4s[:].rearrange("p w -> p w ()"),
                scr[:].rearrange("p (w f) -> p w f", w=NW), axis=X)
            nmr = cst.tile([128, NW], f32)
            nc.scalar.sqrt(nmr[:], s4s[:])
            rinv = cst.tile([128, NW], f32)
            nc.vector.reciprocal(rinv[:], nmr[:])
            nc.vector.tensor_scalar_min(rinv[:], rinv[:], 1.0 / EPS)
            nc.vector.tensor_tensor(
                out=scr[:].rearrange("p (w f) -> p w f", w=NW),
                in0=z_big[:].rearrange("p (w f) -> p w f", w=NW),
                in1=rinv[:].to_broadcast([128, NW, F3]),
                op=ALU.mult)
            # write out: full windows in one strided DMA, tail separate
            nc.sync.dma_start(
                out_t.ap()[0:(NW - 1) * 128, :].rearrange(
                    "(w p) f -> p w f", p=128),
                scr[:].rearrange("p (w f) -> p w f", w=NW)[:, 0:NW - 1, :])
            tail = OWN - (NW - 1) * 128
            nc.sync.dma_start(
                out_t.ap()[(NW - 1) * 128:OWN, :],
                scr[0:tail, (NW - 1) * F3:NW * F3])

    nc.compile()
    return nc


# --------------------------------------------------------------------------
# entry point
# --------------------------------------------------------------------------

def kernel(x, edge_index, W1, b1, W2, b2, W3, b3):
    from concourse.bass_utils import run_bass_kernel_spmd

    x = np.ascontiguousarray(np.asarray(x, dtype=np.float32))
    ei = np.asarray(edge_index)
    src = np.ascontiguousarray(ei[0]).astype(np.int64)
    dst = np.ascontiguousarray(ei[1]).astype(np.int64)
    W1 = np.ascontiguousarray(np.asarray(W1, np.float32)).astype(BF16)
    W2 = np.ascontiguousarray(np.asarray(W2, np.float32)).astype(BF16)
    W3 = np.ascontiguousarray(np.asarray(W3, np.float32)).astype(BF16)
    b1 = np.asarray(b1, np.float32)
    b2 = np.asarray(b2, np.float32)
    b3 = np.asarray(b3, np.float32)
    has_bias = bool(np.any(b1) or np.any(b2) or np.any(b3))

    deg = (np.bincount(dst, minlength=N) + 1.0).astype(np.float32)
    dinv = (1.0 / np.sqrt(deg.astype(np.float64))).astype(np.float32)

    ck = hash((src.tobytes(), dst.tobytes(), has_bias))
    if ck in _BUILD_CACHE:
        nc, sched, per_core = _BUILD_CACHE[ck]
    else:
        sched, per_core = _build_schedule(src, dst)
        nc = _build_nc(sched, has_bias)
        _BUILD_CACHE[ck] = (nc, sched, per_core)

    arange4 = np.tile(np.arange(128, dtype=np.float32)[None, :],
                      (128, 4)).astype(BF16)
    ident = np.eye(128, dtype=np.float32).astype(BF16)
    in_maps = []
    for c in range(NCORES):
        lo = c * OWN
        xT_own = np.zeros((F0, OWN_PAD), BF16)
        xT_own[:, :OWN] = x[lo:lo + OWN].T.astype(BF16)
        dv = np.ones(OWN_PAD, np.float32)
        dv[:OWN] = dinv[lo:lo + OWN]
        dinv_img = np.ascontiguousarray(dv.reshape(NW, 128).T)
        m = {
            "xT_own": xT_own,
            "idxA": per_core[c]["idxA"],
            "idxB": per_core[c]["idxB"],
            "meta": per_core[c]["meta"],
            "dinv_img": dinv_img,
            "arange4": arange4,
            "ident": ident,
            "W1": W1, "W2": W2, "W3": W3,
        }
        if has_bias:
            sq = np.zeros((1, OWN_PAD), np.float32)
            sq[0, :OWN] = np.sqrt(deg[lo:lo + OWN])
            m["b1"] = b1.reshape(1, F1).astype(BF16)
            m["b2"] = b2.reshape(1, F2).astype(BF16)
            m["b3"] = b3.reshape(1, F3).astype(BF16)
            m["sqrtdeg"] = sq.astype(BF16)
        in_maps.append(m)

    res = run_bass_kernel_spmd(nc, in_maps, core_ids=list(range(NCORES)),
                               **_RUN_KWARGS)
    global _LAST
    _LAST = res
    out = np.concatenate([res.results[c]["out"] for c in range(NCORES)], axis=0)
    return out


# test.py hooks (harness never touches these)
_RUN_KWARGS = {}
_LAST = None


# revision 22
# speedup vs baseline: 1.9166x; 1.0918x over previous
"""3-layer GCN (PyG GCNConv-style) Bass/Trainium2 kernel, 8-way SPMD.

Strategy (standard 1D graph partitioning, dst-sharded):
  - Core c owns node rows [c*6250, (c+1)*6250).
  - Per layer: local GEMM H = X@W (PE-transposed activations, W as moving
    operand), scaled by dinv -> Htilde; one AllGather -> full table [50000,F];
    gathers read it as two 25000-row views so indices fit int16.
  - Message passing: dst-sorted edges, bulk dma_gather of source rows,
    aggregated per 128-node window with one-hot "selection" matmuls
    (S[e, n] = 1 iff dst_local[e]==n) accumulating in PSUM; self-loop added
    via identity matmul; out = relu(dinv * psum). S matrices are built 4 per
    DVE op (batched is_equal against broadcast dst_local columns).
  - All tables / messages / weights are bf16 (PSUM accumulates fp32):
    4x PE matmul rate vs fp32, half the gather + AllGather bytes.
  - Schedule (runs per (window, src-half) padded to the max over cores) is
    identical on all cores -> single NEFF; per-core data lives in the
    gather-index / dst-local metadata input tensors.
  - Finals: z kept in SBUF fp32, global sum via AllReduce, z/sum -> tanh^2 ->
    row L2 normalize, all column-batched.
"""

import numpy as np
import ml_dtypes

BF16 = ml_dtypes.bfloat16

# ---- problem constants (hardcoded per contest contract) ----
N = 50000
F0, F1, F2, F3 = 512, 512, 256, 128
NCORES = 8
OWN = N // NCORES            # 6250 rows per core
WIN = 128
NW = (OWN + WIN - 1) // WIN  # 49 windows
OWN_PAD = NW * WIN           # 6272
HALFR = OWN // 2             # 3125: per-rank row split for the 2-chunk AllGather
TBL = NCORES * HALFR         # 25000 rows per gather table (< 32768, int16 ok)
GL = (8, 8, 8)               # gather tiles per dma_gather call, per layer
GATHER_MODE = "queues"         # "plain" | "queues" | "prep" (debug isolation)
CALL = max(GL) * 128         # stream padding granularity (covers all layers)
SENT = 65000.0               # dst_local sentinel -> never matches iota 0..127
EPS = 1e-12

_BUILD_CACHE = {}


# --------------------------------------------------------------------------
# host-side schedule construction (pure index bookkeeping)
# --------------------------------------------------------------------------

def _build_schedule(src, dst):
    """Returns (sched, per_core) where sched is core-independent."""
    src = src.astype(np.int64)
    dst = dst.astype(np.int64)
    core = dst // OWN
    win = (dst % OWN) // WIN
    r = src % OWN
    chunk = (r >= HALFR).astype(np.int64)

    key = (core * NW + win) * 2 + chunk
    order = np.argsort(key, kind="stable")
    counts = np.bincount(key, minlength=NCORES * NW * 2).reshape(NCORES, NW, 2)
    R = counts.max(axis=0)                      # [NW, 2] padded run lengths
    pos = np.zeros((NW, 2), np.int64)           # start position of run (w,c)
    pos[1:, 0] = np.cumsum(R[:-1, 0])
    pos[1:, 1] = np.cumsum(R[:-1, 1])
    slen = R.sum(axis=0)                        # [2] stream lengths
    L = ((slen + CALL - 1) // CALL) * CALL      # padded to gather-call multiple

    # window-of-position per stream (pads extend each run; tail -> -1)
    wof = []
    for c in (0, 1):
        a = np.full(L[c], -1, np.int64)
        a[: slen[c]] = np.repeat(np.arange(NW), R[:, c])
        wof.append(a)

    # pair list: (chunk, tile, meta_col) grouped per window
    window_pairs = [[] for _ in range(NW)]
    mcol = 0
    for w in range(NW):
        for c in (0, 1):
            if R[w, c] == 0:
                continue
            t0 = pos[w, c] // 128
            t1 = (pos[w, c] + R[w, c] - 1) // 128
            for t in range(t0, t1 + 1):
                window_pairs[w].append((c, t, mcol))
                mcol += 1
    TP = mcol

    # per-core gather index streams + meta columns
    per_core = []
    for cc in range(NCORES):
        idx_streams = [np.zeros(L[c], np.int64) for c in (0, 1)]
        dstl_streams = [np.full(L[c], SENT, np.float32) for c in (0, 1)]
        for c in (0, 1):
            sel = order[(core[order] == cc) & (chunk[order] == c)]  # by window
            cnt = counts[cc, :, c]
            starts = pos[:, c]
            within = np.arange(sel.shape[0]) - np.repeat(
                np.concatenate([[0], np.cumsum(cnt[:-1])]), cnt
            )
            p = np.repeat(starts, cnt) + within
            rr = src[sel] % OWN
            tbl_row = (src[sel] // OWN) * HALFR + (rr - c * HALFR)
            idx_streams[c][p] = tbl_row
            dstl_streams[c][p] = (dst[sel] % OWN - win[sel] * WIN).astype(np.float32)
            assert tbl_row.max(initial=0) < TBL

        meta = np.full((128, TP), SENT, np.float32)
        for w in range(NW):
            for c, t, m in window_pairs[w]:
                seg_w = wof[c][t * 128:(t + 1) * 128]
                seg_d = dstl_streams[c][t * 128:(t + 1) * 128]
                meta[:, m] = np.where(seg_w == w, seg_d, SENT)

        imgs = []
        for c in (0, 1):
            a = idx_streams[c].astype(np.int16)
            img = a.reshape(-1, 16).T.copy()          # [16, L/16]
            img = np.tile(img, (8, 1))                # replicate across groups
            imgs.append(np.ascontiguousarray(img))
        per_core.append({"idxA": imgs[0], "idxB": imgs[1],
                         "meta": meta.astype(BF16)})

    sched = {
        "window_pairs": window_pairs,
        "L": [int(L[0]), int(L[1])],
        "TP": TP,
    }
    return sched, per_core


# --------------------------------------------------------------------------
# device kernel builder
# --------------------------------------------------------------------------

def _build_nc(sched, has_bias):
    import concourse.bacc as bacc
    import concourse.mybir as mybir
    import concourse.tile as tile

    f32 = mybir.dt.float32
    bf16 = mybir.dt.bfloat16
    i16 = mybir.dt.int16
    AF = mybir.ActivationFunctionType
    ALU = mybir.AluOpType
    X = mybir.AxisListType.X
    RG = [list(range(NCORES))]

    LA, LB = sched["L"]
    TP = sched["TP"]
    window_pairs = sched["window_pairs"]

    nc = bacc.Bacc("TRN2", target_bir_lowering=False, debug=False,
                   num_devices=NCORES, num_swdge_queues=4)

    xT_t = nc.dram_tensor("xT_own", [F0, OWN_PAD], bf16, kind="ExternalInput")
    idxA_t = nc.dram_tensor("idxA", [128, LA // 16], i16, kind="ExternalInput")
    idxB_t = nc.dram_tensor("idxB", [128, LB // 16], i16, kind="ExternalInput")
    meta_t = nc.dram_tensor("meta", [128, TP], bf16, kind="ExternalInput")
    dinv_t = nc.dram_tensor("dinv_img", [128, NW], f32, kind="ExternalInput")
    ar_t = nc.dram_tensor("arange4", [128, 512], bf16, kind="ExternalInput")
    id_t = nc.dram_tensor("ident", [128, 128], bf16, kind="ExternalInput")
    w1_t = nc.dram_tensor("W1", [F0, F1], bf16, kind="ExternalInput")
    w2_t = nc.dram_tensor("W2", [F1, F2], bf16, kind="ExternalInput")
    w3_t = nc.dram_tensor("W3", [F2, F3], bf16, kind="ExternalInput")
    if has_bias:
        b1_t = nc.dram_tensor("b1", [1, F1], bf16, kind="ExternalInput")
        b2_t = nc.dram_tensor("b2", [1, F2], bf16, kind="ExternalInput")
        b3_t = nc.dram_tensor("b3", [1, F3], bf16, kind="ExternalInput")
        sqd_t = nc.dram_tensor("sqrtdeg", [1, OWN_PAD], bf16, kind="ExternalInput")
    out_t = nc.dram_tensor("out", [OWN, F3], f32, kind="ExternalOutput")

    with tile.TileContext(nc) as tc:
        with (
            tc.tile_pool(name="dram", bufs=1, space="DRAM") as dram,
            tc.tile_pool(name="const", bufs=1) as cst,
            tc.tile_pool(name="sb", bufs=2) as sb,
            tc.tile_pool(name="spool", bufs=6) as sp,
            tc.tile_pool(name="ps", bufs=2, space="PSUM") as ps,
            tc.tile_pool(name="ps1", bufs=1, space="PSUM") as ps1,
        ):
            # ---- resident constants ----
            ar_sb = cst.tile([128, 512], bf16)
            nc.sync.dma_start(ar_sb[:], ar_t.ap())
            id_sb = cst.tile([128, 128], bf16)
            nc.sync.dma_start(id_sb[:], id_t.ap())
            dinv_sb = cst.tile([128, NW], f32)
            nc.sync.dma_start(dinv_sb[:], dinv_t.ap())
            meta_sb = cst.tile([128, TP], bf16)
            nc.sync.dma_start(meta_sb[:], meta_t.ap())
            idx_sb = []
            for name, t_, Lc in (("ia", idxA_t, LA), ("ib", idxB_t, LB)):
                tl = cst.tile([128, Lc // 16], i16, name=name)
                nc.sync.dma_start(tl[:], t_.ap())
                idx_sb.append(tl)
            w_sb = []
            for name, t_, fi, fo in (("w1", w1_t, F0, F1), ("w2", w2_t, F1, F2),
                                     ("w3", w3_t, F2, F3)):
                kt = fi // 128
                tl = cst.tile([128, kt * fo], bf16, name=name)
                nc.sync.dma_start(
                    tl[:].rearrange("p (k f) -> p k f", k=kt),
                    t_.ap().rearrange("(k p) f -> p k f", p=128))
                w_sb.append(tl)
            b_sb = []
            sqd_sb = None
            if has_bias:
                for name, t_, fo in (("b1s", b1_t, F1), ("b2s", b2_t, F2),
                                     ("b3s", b3_t, F3)):
                    tl = cst.tile([1, fo], bf16, name=name)
                    nc.sync.dma_start(tl[:], t_.ap())
                    b_sb.append(tl)
                sqd_sb = cst.tile([1, OWN_PAD], bf16)
                nc.sync.dma_start(sqd_sb[:], sqd_t.ap())
            z_big = cst.tile([128, NW * F3], f32)

            # ---- DRAM intermediates ----
            agA = [dram.tile([HALFR, f], bf16, name=f"agA{i}")
                   for i, f in enumerate((F1, F2, F3))]
            agB = [dram.tile([OWN_PAD - HALFR, f], bf16, name=f"agB{i}")
                   for i, f in enumerate((F1, F2, F3))]
            tblA = [dram.tile([TBL, f], bf16, name=f"tA{i}", addr_space="Shared")
                    for i, f in enumerate((F1, F2, F3))]
            tblB = [dram.tile([TBL, f], bf16, name=f"tB{i}", addr_space="Shared")
                    for i, f in enumerate((F1, F2, F3))]
            BW = HALFR // 128           # full windows in the A chunk
            BCUT = HALFR - BW * 128     # rows of the boundary window in A

            def dinv_col(w):
                return dinv_sb[:, w:w + 1]

            def write_h(hb, w, li):
                a, b = agA[li], agB[li]
                if w < BW:
                    nc.sync.dma_start(a[w * 128:(w + 1) * 128, :], hb[:])
                elif w == BW and BCUT > 0:
                    nc.sync.dma_start(a[BW * 128:HALFR, :], hb[:BCUT, :])
                    nc.sync.dma_start(b[0:128 - BCUT, :], hb[BCUT:, :])
                else:
                    o = w * 128 - HALFR
                    nc.sync.dma_start(b[o:o + 128, :], hb[:])

            def read_own(ob, w, li):
                a, b = agA[li], agB[li]
                if w < BW:
                    nc.sync.dma_start(ob[:], a[w * 128:(w + 1) * 128, :])
                elif w == BW and BCUT > 0:
                    nc.sync.dma_start(ob[:BCUT, :], a[BW * 128:HALFR, :])
                    nc.sync.dma_start(ob[BCUT:, :], b[0:128 - BCUT, :])
                else:
                    o = w * 128 - HALFR
                    nc.sync.dma_start(ob[:], b[o:o + 128, :])

            def emit_agA(li):
                nc.gpsimd.collective_compute(
                    "AllGather", ALU.bypass, replica_groups=RG,
                    ins=[agA[li][:].opt()], outs=[tblA[li][:].opt()])

            def emit_agB(li):
                nc.gpsimd.collective_compute(
                    "AllGather", ALU.bypass, replica_groups=RG,
                    ins=[agB[li][0:OWN - HALFR, :].opt()],
                    outs=[tblB[li][:].opt()])

            # ---- GEMM for one 128-row block (node-major in and out) ----
            def gemm_block(blk, w, fi, fo, wsb, li_next):
                kt = fi // 128
                psg = ps.tile([128, fo], f32, name="psg", tag="psg")
                for k in range(kt):
                    pst = ps.tile([128, 128], bf16, name="pst", tag="pst")
                    nc.tensor.transpose(pst[:], blk[:, k * 128:(k + 1) * 128],
                                        id_sb[:])
                    hT = sb.tile([128, 128], bf16, name="hT", tag="hT", bufs=8)
                    nc.scalar.copy(hT[:], pst[:])
                    nc.tensor.matmul(psg[:], lhsT=hT[:],
                                     rhs=wsb[:, k * fo:(k + 1) * fo],
                                     start=(k == 0), stop=(k == kt - 1))
                hb = sb.tile([128, fo], bf16, name="hb", tag="hb")
                nc.scalar.mul(hb[:], psg[:], dinv_col(w))
                write_h(hb, w, li_next)

            # ---- phase 0: layer-1 GEMM over own rows (x pre-transposed) ----
            # x slabs loaded whole (4 big DMAs) into a scoped pool released
            # before the gather-message pool opens.
            kt0 = F0 // 128
            with tc.tile_pool(name="xsl", bufs=1) as xp:
                xsl = []
                for k in range(kt0):
                    t = xp.tile([128, OWN_PAD], bf16, name=f"xs{k}")
                    nc.sync.dma_start(t[:], xT_t.ap()[k * 128:(k + 1) * 128, :])
                    xsl.append(t)
                for w in range(NW):
                    psg = ps.tile([128, F1], f32, name="psg", tag="psg")
                    for k in range(kt0):
                        nc.tensor.matmul(psg[:],
                                         lhsT=xsl[k][:, w * 128:(w + 1) * 128],
                                         rhs=w_sb[0][:, k * F1:(k + 1) * F1],
                                         start=(k == 0), stop=(k == kt0 - 1))
                    hb = sb.tile([128, F1], bf16, name="hb", tag="hb")
                    nc.scalar.mul(hb[:], psg[:], dinv_col(w))
                    write_h(hb, w, 0)
                    if w == (BW if BCUT > 0 else BW - 1):
                        emit_agA(0)
                emit_agB(0)

            # ---- layers ----
            # Gathers are PREPARE_ONLY on two SWDGE queues (chunk A -> q0,
            # chunk B -> q1): descriptor generation (the serial Q7 cost) never
            # waits on an AllGather; the cheap trigger_dma carries the
            # table-ready dependency instead. Each prep must bake in the
            # DMASW-lane global sem Tile's wait pass will reference —
            # lanes round-robin over every SWDGE DMA in engine order.
            lane_sems = tc.sems.swdge_block()
            prep_count = [0]
            LOOK = 4
            with tc.tile_pool(name="mpool", bufs=6) as mp:
                for li, fo in enumerate((F1, F2, F3)):
                    tviews = (tblA[li][:], tblB[li][:])

                    live = [{}, {}]
                    prepped = [0, 0]
                    triggered = [0, 0]
                    s4_live = {}
                    s4_next = [0]
                    Gl = GL[li]
                    CALLl = Gl * 128
                    ncalls = [LA // CALLl, LB // CALLl]

                    def prep(c, hi, fo=fo, tviews=tviews, live=live,
                             prepped=prepped, Gl=Gl, CALLl=CALLl,
                             ncalls=ncalls):
                        hi = min(hi, ncalls[c] - 1)
                        while prepped[c] <= hi:
                            ci = prepped[c]
                            m = mp.tile([128, Gl * fo], bf16, name=f"m{c}",
                                        tag=f"m{c}")
                            kw = {}
                            if GATHER_MODE == "prep":
                                kw = dict(prepare_only=True,
                                          sem=lane_sems[prep_count[0]
                                                        % len(lane_sems)])
                                prep_count[0] += 1
                            elif GATHER_MODE == "queues":
                                kw = dict(queue_num=c + 2 * (ci % 2))
                            nc.gpsimd.dma_gather(
                                m[:].rearrange("p (t f) -> p t f", f=fo),
                                tviews[c],
                                idx_sb[c][:, ci * (CALLl // 16):
                                          (ci + 1) * (CALLl // 16)],
                                CALLl, CALLl, fo, **kw)
                            live[c][ci] = m
                            prepped[c] += 1

                    def ensure(c, t, live=live, prepped=prepped,
                               triggered=triggered, Gl=Gl, ncalls=ncalls,
                               prep=prep):
                        call = t // Gl
                        if triggered[c] <= call:
                            prep(c, call + LOOK)
                            if GATHER_MODE == "prep":
                                # one shared queue: a trigger drains BOTH
                                # streams' pending preps
                                if (prepped[0] > triggered[0]
                                        or prepped[1] > triggered[1]):
                                    nc.gpsimd.trigger_dma(count=None)
                                triggered[0] = prepped[0]
                                triggered[1] = prepped[1]
                            else:
                                triggered[c] = prepped[c]
                        return live[c][call]

                    def ensure_s4(mcol, s4_live=s4_live, s4_next=s4_next):
                        b = mcol // 4
                        while s4_next[0] <= b:
                            bi = s4_next[0]
                            m0 = bi * 4
                            nb = min(4, TP - m0)
                            s4 = sp.tile([128, 512], bf16, name="s4", tag="s4")
                            nc.vector.tensor_tensor(
                                out=s4[:].rearrange(
                                    "p (a f) -> p a f", a=4)[:, :nb, :],
                                in0=ar_sb[:].rearrange(
                                    "p (a f) -> p a f", a=4)[:, :nb, :],
                                in1=meta_sb[:, m0:m0 + nb].to_broadcast(
                                    [128, nb, 128]),
                                op=ALU.is_equal)
                            s4_live[bi] = s4
                            s4_next[0] += 1
                        return s4_live[b]

                    # desc-gen warm-up: queue LOOK+1 calls deep on both
                    # streams before the first window consumes anything.
                    prep(0, LOOK)
                    prep(1, LOOK)

                    for w in range(NW):
                        psw = ps.tile([128, fo], f32, name="psw", tag="psw")
                        first = True
                        for c, t, mcolv in window_pairs[w]:
                            m = ensure(c, t)
                            s4 = ensure_s4(mcolv)
                            j = mcolv % 4
                            sl = t % Gl
                            nc.tensor.matmul(psw[:],
                                             lhsT=s4[:, j * 128:(j + 1) * 128],
                                             rhs=m[:, sl * fo:(sl + 1) * fo],
                                             start=first, stop=False)
                            first = False
                        ob = sb.tile([128, fo], bf16, name="ob", tag="ob")
                        read_own(ob, w, li)
                        stop_here = not has_bias
                        nc.tensor.matmul(psw[:], lhsT=id_sb[:], rhs=ob[:],
                                         start=first, stop=stop_here)
                        if has_bias:
                            nc.tensor.matmul(
                                psw[:],
                                lhsT=sqd_sb[0:1, w * 128:(w + 1) * 128],
                                rhs=b_sb[li][0:1, :], start=False, stop=True)
                        if li < 2:
                            hb = sb.tile([128, fo], bf16, name="hbw",
                                         tag="hbw")
                            nc.scalar.activation(hb[:], psw[:], AF.Relu,
                                                 bias=0.0, scale=dinv_col(w))
                            gemm_block(hb, w, fo, (F2, F3)[li], w_sb[li + 1],
                                       li + 1)
                            if w == (BW if BCUT > 0 else BW - 1):
                                emit_agA(li + 1)
                        else:
                            nc.scalar.activation(
                                z_big[:, w * F3:(w + 1) * F3], psw[:],
                                AF.Relu, bias=0.0, scale=dinv_col(w))
                    if li < 2:
                        emit_agB(li + 1)

            # ---- finals ----
            zsum = cst.tile([128, 1], f32)
            nc.vector.reduce_sum(zsum[:], z_big[:], axis=X)
            ones = cst.tile([128, 1], f32)
            nc.vector.memset(ones[:], 1.0)
            pss = ps1.tile([128, 16], f32)
            nc.tensor.matmul(pss[0:1, 0:1], lhsT=ones[:], rhs=zsum[:],
                             start=True, stop=True)
            tot_sb = cst.tile([1, 16], f32)
            nc.vector.memset(tot_sb[:], 0.0)
            nc.scalar.copy(tot_sb[0:1, 0:1], pss[0:1, 0:1])
            ar_in = dram.tile([1, 16], f32)
            ar_out = dram.tile([1, 16], f32, addr_space="Shared")
            nc.sync.dma_start(ar_in[:], tot_sb[:])
            nc.gpsimd.collective_compute(
                "AllReduce", ALU.add, replica_groups=RG,
                ins=[ar_in[:].opt()], outs=[ar_out[:].opt()])
            tot2 = cst.tile([1, 16], f32)
            nc.sync.dma_start(tot2[:], ar_out[:])
            tot_bc = cst.tile([128, 1], f32)
            nc.gpsimd.partition_broadcast(tot_bc[:], tot2[0:1, 0:1],
                                          channels=128)
            inv_tot = cst.tile([128, 1], f32)
            nc.vector.reciprocal(inv_tot[:], tot_bc[:])

            # z/sum -> tanh -> ^2 ; then row L2 norm, all column-batched.
            # Ping-pong z_big <-> scr to bound SBUF: scr=tanh(z);
            # z_big=tanh^2; scr=tanh^4; reduce; scr=z_big*rinv; DMA out.
            scr = cst.tile([128, NW * F3], f32)
            nc.scalar.activation(scr[:], z_big[:], AF.Tanh, bias=0.0,
                                 scale=inv_tot[:])
            nc.scalar.square(z_big[:], scr[:])
            nc.scalar.square(scr[:], z_big[:])
            s4s = cst.tile([128, NW], f32)
            nc.vector.reduce_sum(
                s4s[:].rearrange("p w -> p w ()"),
                scr[:].rearrange("p (w f) -> p w f", w=NW), axis=X)
            nmr = cst.tile([128, NW], f32)
            nc.scalar.sqrt(nmr[:], s4s[:])
            rinv = cst.tile([128, NW], f32)
            nc.vector.reciprocal(rinv[:], nmr[:])
            nc.vector.tensor_scalar_min(rinv[:], rinv[:], 1.0 / EPS)
            nc.vector.tensor_tensor(
                out=scr[:].rearrange("p (w f) -> p w f", w=NW),
                in0=z_big[:].rearrange("p (w f) -> p w f", w=NW),
                in1=rinv[:].to_broadcast([128, NW, F3]),
                op=ALU.mult)
            # write out: full windows in one strided DMA, tail separate
            nc.sync.dma_start(
                out_t.ap()[0:(NW - 1) * 128, :].rearrange(
                    "(w p) f -> p w f", p=128),
                scr[:].rearrange("p (w f) -> p w f", w=NW)[:, 0:NW - 1, :])
            tail = OWN - (NW - 1) * 128
            nc.sync.dma_start(
                out_t.ap()[(NW - 1) * 128:OWN, :],
                scr[0:tail, (NW - 1) * F3:NW * F3])

    nc.compile()
    return nc


# --------------------------------------------------------------------------
# entry point
# --------------------------------------------------------------------------

def kernel(x, edge_index, W1, b1, W2, b2, W3, b3):
    from concourse.bass_utils import run_bass_kernel_spmd

    x = np.ascontiguousarray(np.asarray(x, dtype=np.float32))
    ei = np.asarray(edge_index)
    src = np.ascontiguousarray(ei[0]).astype(np.int64)
    dst = np.ascontiguousarray(ei[1]).astype(np.int64)
    W1 = np.ascontiguousarray(np.asarray(W1, np.float32)).astype(BF16)
    W2 = np.ascontiguousarray(np.asarray(W2, np.float32)).astype(BF16)
    W3 = np.ascontiguousarray(np.asarray(W3, np.float32)).astype(BF16)
    b1 = np.asarray(b1, np.float32)
    b2 = np.asarray(b2, np.float32)
    b3 = np.asarray(b3, np.float32)
    has_bias = bool(np.any(b1) or np.any(b2) or np.any(b3))

    deg = (np.bincount(dst, minlength=N) + 1.0).astype(np.float32)
    dinv = (1.0 / np.sqrt(deg.astype(np.float64))).astype(np.float32)

    ck = hash((src.tobytes(), dst.tobytes(), has_bias))
    if ck in _BUILD_CACHE:
        nc, sched, per_core = _BUILD_CACHE[ck]
    else:
        sched, per_core = _build_schedule(src, dst)
        nc = _build_nc(sched, has_bias)
        _BUILD_CACHE[ck] = (nc, sched, per_core)

    arange4 = np.tile(np.arange(128, dtype=np.float32)[None, :],
                      (128, 4)).astype(BF16)
    ident = np.eye(128, dtype=np.float32).astype(BF16)
    in_maps = []
    for c in range(NCORES):
        lo = c * OWN
        xT_own = np.zeros((F0, OWN_PAD), BF16)
        xT_own[:, :OWN] = x[lo:lo + OWN].T.astype(BF16)
        dv = np.ones(OWN_PAD, np.float32)
        dv[:OWN] = dinv[lo:lo + OWN]
        dinv_img = np.ascontiguousarray(dv.reshape(NW, 128).T)
        m = {
            "xT_own": xT_own,
            "idxA": per_core[c]["idxA"],
            "idxB": per_core[c]["idxB"],
            "meta": per_core[c]["meta"],
            "dinv_img": dinv_img,
            "arange4": arange4,
            "ident": ident,
            "W1": W1, "W2": W2, "W3": W3,
        }
        if has_bias:
            sq = np.zeros((1, OWN_PAD), np.float32)
            sq[0, :OWN] = np.sqrt(deg[lo:lo + OWN])
            m["b1"] = b1.reshape(1, F1).astype(BF16)
            m["b2"] = b2.reshape(1, F2).astype(BF16)
            m["b3"] = b3.reshape(1, F3).astype(BF16)
            m["sqrtdeg"] = sq.astype(BF16)
        in_maps.append(m)

    res = run_bass_kernel_spmd(nc, in_maps, core_ids=list(range(NCORES)),
                               **_RUN_KWARGS)
    global _LAST
    _LAST = res
    out = np.concatenate([res.results[c]["out"] for c in range(NCORES)], axis=0)
    return out


# test.py hooks (harness never touches these)
_RUN_KWARGS = {}
_LAST = None


# revision 23
# speedup vs baseline: 1.9362x; 1.0102x over previous
"""3-layer GCN (PyG GCNConv-style) Bass/Trainium2 kernel, 8-way SPMD.

Strategy (standard 1D graph partitioning, dst-sharded):
  - Core c owns node rows [c*6250, (c+1)*6250).
  - Per layer: local GEMM H = X@W (PE-transposed activations, W as moving
    operand), scaled by dinv -> Htilde; one AllGather -> full table [50000,F];
    gathers read it as two 25000-row views so indices fit int16.
  - Message passing: dst-sorted edges, bulk dma_gather of source rows,
    aggregated per 128-node window with one-hot "selection" matmuls
    (S[e, n] = 1 iff dst_local[e]==n) accumulating in PSUM; self-loop added
    via identity matmul; out = relu(dinv * psum). S matrices are built 4 per
    DVE op (batched is_equal against broadcast dst_local columns).
  - All tables / messages / weights are bf16 (PSUM accumulates fp32):
    4x PE matmul rate vs fp32, half the gather + AllGather bytes.
  - Schedule (runs per (window, src-half) padded to the max over cores) is
    identical on all cores -> single NEFF; per-core data lives in the
    gather-index / dst-local metadata input tensors.
  - Finals: z kept in SBUF fp32, global sum via AllReduce, z/sum -> tanh^2 ->
    row L2 normalize, all column-batched.
"""

import numpy as np
import ml_dtypes

BF16 = ml_dtypes.bfloat16

# ---- problem constants (hardcoded per contest contract) ----
N = 50000
F0, F1, F2, F3 = 512, 512, 256, 128
NCORES = 8
OWN = N // NCORES            # 6250 rows per core
WIN = 128
NW = (OWN + WIN - 1) // WIN  # 49 windows
OWN_PAD = NW * WIN           # 6272
HALFR = OWN // 2             # 3125: per-rank row split for the 2-chunk AllGather
TBL = NCORES * HALFR         # 25000 rows per gather table (< 32768, int16 ok)
GL = (8, 8, 8)               # gather tiles per dma_gather call, per layer
GATHER_MODE = "queues"         # "plain" | "queues" | "prep" (debug isolation)
CALL = max(GL) * 128         # stream padding granularity (covers all layers)
SENT = 65000.0               # dst_local sentinel -> never matches iota 0..127
EPS = 1e-12

_BUILD_CACHE = {}


# --------------------------------------------------------------------------
# host-side schedule construction (pure index bookkeeping)
# --------------------------------------------------------------------------

def _build_schedule(src, dst):
    """Returns (sched, per_core) where sched is core-independent."""
    src = src.astype(np.int64)
    dst = dst.astype(np.int64)
    core = dst // OWN
    win = (dst % OWN) // WIN
    r = src % OWN
    chunk = (r >= HALFR).astype(np.int64)

    key = (core * NW + win) * 2 + chunk
    order = np.argsort(key, kind="stable")
    counts = np.bincount(key, minlength=NCORES * NW * 2).reshape(NCORES, NW, 2)
    R = counts.max(axis=0)                      # [NW, 2] padded run lengths
    pos = np.zeros((NW, 2), np.int64)           # start position of run (w,c)
    pos[1:, 0] = np.cumsum(R[:-1, 0])
    pos[1:, 1] = np.cumsum(R[:-1, 1])
    slen = R.sum(axis=0)                        # [2] stream lengths
    L = ((slen + CALL - 1) // CALL) * CALL      # padded to gather-call multiple

    # window-of-position per stream (pads extend each run; tail -> -1)
    wof = []
    for c in (0, 1):
        a = np.full(L[c], -1, np.int64)
        a[: slen[c]] = np.repeat(np.arange(NW), R[:, c])
        wof.append(a)

    # pair list: (chunk, tile, meta_col) grouped per window
    window_pairs = [[] for _ in range(NW)]
    mcol = 0
    for w in range(NW):
        for c in (0, 1):
            if R[w, c] == 0:
                continue
            t0 = pos[w, c] // 128
            t1 = (pos[w, c] + R[w, c] - 1) // 128
            for t in range(t0, t1 + 1):
                window_pairs[w].append((c, t, mcol))
                mcol += 1
    TP = mcol

    # per-core gather index streams + meta columns
    per_core = []
    for cc in range(NCORES):
        idx_streams = [np.zeros(L[c], np.int64) for c in (0, 1)]
        dstl_streams = [np.full(L[c], SENT, np.float32) for c in (0, 1)]
        for c in (0, 1):
            sel = order[(core[order] == cc) & (chunk[order] == c)]  # by window
            cnt = counts[cc, :, c]
            starts = pos[:, c]
            within = np.arange(sel.shape[0]) - np.repeat(
                np.concatenate([[0], np.cumsum(cnt[:-1])]), cnt
            )
            p = np.repeat(starts, cnt) + within
            rr = src[sel] % OWN
            tbl_row = (src[sel] // OWN) * HALFR + (rr - c * HALFR)
            idx_streams[c][p] = tbl_row
            dstl_streams[c][p] = (dst[sel] % OWN - win[sel] * WIN).astype(np.float32)
            assert tbl_row.max(initial=0) < TBL

        meta = np.full((128, TP), SENT, np.float32)
        for w in range(NW):
            for c, t, m in window_pairs[w]:
                seg_w = wof[c][t * 128:(t + 1) * 128]
                seg_d = dstl_streams[c][t * 128:(t + 1) * 128]
                meta[:, m] = np.where(seg_w == w, seg_d, SENT)

        imgs = []
        for c in (0, 1):
            a = idx_streams[c].astype(np.int16)
            img = a.reshape(-1, 16).T.copy()          # [16, L/16]
            img = np.tile(img, (8, 1))                # replicate across groups
            imgs.append(np.ascontiguousarray(img))
        per_core.append({"idxA": imgs[0], "idxB": imgs[1],
                         "meta": meta.astype(BF16)})

    sched = {
        "window_pairs": window_pairs,
        "L": [int(L[0]), int(L[1])],
        "TP": TP,
    }
    return sched, per_core


# --------------------------------------------------------------------------
# device kernel builder
# --------------------------------------------------------------------------

def _build_nc(sched, has_bias):
    import concourse.bacc as bacc
    import concourse.mybir as mybir
    import concourse.tile as tile

    f32 = mybir.dt.float32
    bf16 = mybir.dt.bfloat16
    i16 = mybir.dt.int16
    AF = mybir.ActivationFunctionType
    ALU = mybir.AluOpType
    X = mybir.AxisListType.X
    RG = [list(range(NCORES))]

    LA, LB = sched["L"]
    TP = sched["TP"]
    window_pairs = sched["window_pairs"]

    nc = bacc.Bacc("TRN2", target_bir_lowering=False, debug=False,
                   num_devices=NCORES, num_swdge_queues=4)

    xT_t = nc.dram_tensor("xT_own", [F0, OWN_PAD], bf16, kind="ExternalInput")
    idxA_t = nc.dram_tensor("idxA", [128, LA // 16], i16, kind="ExternalInput")
    idxB_t = nc.dram_tensor("idxB", [128, LB // 16], i16, kind="ExternalInput")
    meta_t = nc.dram_tensor("meta", [128, TP], bf16, kind="ExternalInput")
    dinv_t = nc.dram_tensor("dinv_img", [128, NW], f32, kind="ExternalInput")
    ar_t = nc.dram_tensor("arange4", [128, 512], bf16, kind="ExternalInput")
    id_t = nc.dram_tensor("ident", [128, 128], bf16, kind="ExternalInput")
    w1_t = nc.dram_tensor("W1", [F0, F1], bf16, kind="ExternalInput")
    w2_t = nc.dram_tensor("W2", [F1, F2], bf16, kind="ExternalInput")
    w3_t = nc.dram_tensor("W3", [F2, F3], bf16, kind="ExternalInput")
    if has_bias:
        b1_t = nc.dram_tensor("b1", [1, F1], bf16, kind="ExternalInput")
        b2_t = nc.dram_tensor("b2", [1, F2], bf16, kind="ExternalInput")
        b3_t = nc.dram_tensor("b3", [1, F3], bf16, kind="ExternalInput")
        sqd_t = nc.dram_tensor("sqrtdeg", [1, OWN_PAD], bf16, kind="ExternalInput")
    out_t = nc.dram_tensor("out", [OWN, F3], f32, kind="ExternalOutput")

    with tile.TileContext(nc) as tc:
        with (
            tc.tile_pool(name="dram", bufs=1, space="DRAM") as dram,
            tc.tile_pool(name="const", bufs=1) as cst,
            tc.tile_pool(name="sb", bufs=2) as sb,
            tc.tile_pool(name="spool", bufs=6) as sp,
            tc.tile_pool(name="ps", bufs=2, space="PSUM") as ps,
            tc.tile_pool(name="ps1", bufs=1, space="PSUM") as ps1,
        ):
            # ---- resident constants ----
            ar_sb = cst.tile([128, 512], bf16)
            nc.sync.dma_start(ar_sb[:], ar_t.ap())
            id_sb = cst.tile([128, 128], bf16)
            nc.sync.dma_start(id_sb[:], id_t.ap())
            dinv_sb = cst.tile([128, NW], f32)
            nc.sync.dma_start(dinv_sb[:], dinv_t.ap())
            meta_sb = cst.tile([128, TP], bf16)
            nc.sync.dma_start(meta_sb[:], meta_t.ap())
            idx_sb = []
            for name, t_, Lc in (("ia", idxA_t, LA), ("ib", idxB_t, LB)):
                tl = cst.tile([128, Lc // 16], i16, name=name)
                nc.sync.dma_start(tl[:], t_.ap())
                idx_sb.append(tl)
            w_sb = []
            for name, t_, fi, fo in (("w1", w1_t, F0, F1), ("w2", w2_t, F1, F2),
                                     ("w3", w3_t, F2, F3)):
                kt = fi // 128
                tl = cst.tile([128, kt * fo], bf16, name=name)
                nc.sync.dma_start(
                    tl[:].rearrange("p (k f) -> p k f", k=kt),
                    t_.ap().rearrange("(k p) f -> p k f", p=128))
                w_sb.append(tl)
            b_sb = []
            sqd_sb = None
            if has_bias:
                for name, t_, fo in (("b1s", b1_t, F1), ("b2s", b2_t, F2),
                                     ("b3s", b3_t, F3)):
                    tl = cst.tile([1, fo], bf16, name=name)
                    nc.sync.dma_start(tl[:], t_.ap())
                    b_sb.append(tl)
                sqd_sb = cst.tile([1, OWN_PAD], bf16)
                nc.sync.dma_start(sqd_sb[:], sqd_t.ap())
            z_big = cst.tile([128, NW * F3], f32)
            zpart = cst.tile([128, NW], f32)

            # ---- DRAM intermediates ----
            agA = [dram.tile([HALFR, f], bf16, name=f"agA{i}")
                   for i, f in enumerate((F1, F2, F3))]
            agB = [dram.tile([OWN_PAD - HALFR, f], bf16, name=f"agB{i}")
                   for i, f in enumerate((F1, F2, F3))]
            tblA = [dram.tile([TBL, f], bf16, name=f"tA{i}", addr_space="Shared")
                    for i, f in enumerate((F1, F2, F3))]
            tblB = [dram.tile([TBL, f], bf16, name=f"tB{i}", addr_space="Shared")
                    for i, f in enumerate((F1, F2, F3))]
            BW = HALFR // 128           # full windows in the A chunk
            BCUT = HALFR - BW * 128     # rows of the boundary window in A

            def dinv_col(w):
                return dinv_sb[:, w:w + 1]

            def write_h(hb, w, li):
                a, b = agA[li], agB[li]
                if w < BW:
                    nc.sync.dma_start(a[w * 128:(w + 1) * 128, :], hb[:])
                elif w == BW and BCUT > 0:
                    nc.sync.dma_start(a[BW * 128:HALFR, :], hb[:BCUT, :])
                    nc.sync.dma_start(b[0:128 - BCUT, :], hb[BCUT:, :])
                else:
                    o = w * 128 - HALFR
                    nc.sync.dma_start(b[o:o + 128, :], hb[:])

            def read_own(ob, w, li):
                a, b = agA[li], agB[li]
                if w < BW:
                    nc.sync.dma_start(ob[:], a[w * 128:(w + 1) * 128, :])
                elif w == BW and BCUT > 0:
                    nc.sync.dma_start(ob[:BCUT, :], a[BW * 128:HALFR, :])
                    nc.sync.dma_start(ob[BCUT:, :], b[0:128 - BCUT, :])
                else:
                    o = w * 128 - HALFR
                    nc.sync.dma_start(ob[:], b[o:o + 128, :])

            def emit_agA(li):
                nc.gpsimd.collective_compute(
                    "AllGather", ALU.bypass, replica_groups=RG,
                    ins=[agA[li][:].opt()], outs=[tblA[li][:].opt()])

            def emit_agB(li):
                nc.gpsimd.collective_compute(
                    "AllGather", ALU.bypass, replica_groups=RG,
                    ins=[agB[li][0:OWN - HALFR, :].opt()],
                    outs=[tblB[li][:].opt()])

            # ---- GEMM for one 128-row block (node-major in and out) ----
            def gemm_block(blk, w, fi, fo, wsb, li_next):
                kt = fi // 128
                psg = ps.tile([128, fo], f32, name="psg", tag="psg")
                for k in range(kt):
                    pst = ps.tile([128, 128], bf16, name="pst", tag="pst")
                    nc.tensor.transpose(pst[:], blk[:, k * 128:(k + 1) * 128],
                                        id_sb[:])
                    hT = sb.tile([128, 128], bf16, name="hT", tag="hT", bufs=8)
                    nc.scalar.copy(hT[:], pst[:])
                    nc.tensor.matmul(psg[:], lhsT=hT[:],
                                     rhs=wsb[:, k * fo:(k + 1) * fo],
                                     start=(k == 0), stop=(k == kt - 1))
                hb = sb.tile([128, fo], bf16, name="hb", tag="hb")
                nc.scalar.mul(hb[:], psg[:], dinv_col(w))
                write_h(hb, w, li_next)

            # ---- phase 0: layer-1 GEMM over own rows (x pre-transposed) ----
            # x slabs loaded whole (4 big DMAs) into a scoped pool released
            # before the gather-message pool opens.
            kt0 = F0 // 128
            with tc.tile_pool(name="xsl", bufs=1) as xp:
                xsl = []
                for k in range(kt0):
                    t = xp.tile([128, OWN_PAD], bf16, name=f"xs{k}")
                    nc.sync.dma_start(t[:], xT_t.ap()[k * 128:(k + 1) * 128, :])
                    xsl.append(t)
                for w in range(NW):
                    psg = ps.tile([128, F1], f32, name="psg", tag="psg")
                    for k in range(kt0):
                        nc.tensor.matmul(psg[:],
                                         lhsT=xsl[k][:, w * 128:(w + 1) * 128],
                                         rhs=w_sb[0][:, k * F1:(k + 1) * F1],
                                         start=(k == 0), stop=(k == kt0 - 1))
                    hb = sb.tile([128, F1], bf16, name="hb", tag="hb")
                    nc.scalar.mul(hb[:], psg[:], dinv_col(w))
                    write_h(hb, w, 0)
                    if w == (BW if BCUT > 0 else BW - 1):
                        emit_agA(0)
                emit_agB(0)

            # ---- layers ----
            # Gathers are PREPARE_ONLY on two SWDGE queues (chunk A -> q0,
            # chunk B -> q1): descriptor generation (the serial Q7 cost) never
            # waits on an AllGather; the cheap trigger_dma carries the
            # table-ready dependency instead. Each prep must bake in the
            # DMASW-lane global sem Tile's wait pass will reference —
            # lanes round-robin over every SWDGE DMA in engine order.
            lane_sems = tc.sems.swdge_block()
            prep_count = [0]
            LOOK = 4
            with tc.tile_pool(name="mpool", bufs=6) as mp:
                for li, fo in enumerate((F1, F2, F3)):
                    tviews = (tblA[li][:], tblB[li][:])

                    live = [{}, {}]
                    prepped = [0, 0]
                    triggered = [0, 0]
                    s4_live = {}
                    s4_next = [0]
                    Gl = GL[li]
                    CALLl = Gl * 128
                    ncalls = [LA // CALLl, LB // CALLl]

                    def prep(c, hi, fo=fo, tviews=tviews, live=live,
                             prepped=prepped, Gl=Gl, CALLl=CALLl,
                             ncalls=ncalls):
                        hi = min(hi, ncalls[c] - 1)
                        while prepped[c] <= hi:
                            ci = prepped[c]
                            m = mp.tile([128, Gl * fo], bf16, name=f"m{c}",
                                        tag=f"m{c}")
                            kw = {}
                            if GATHER_MODE == "prep":
                                kw = dict(prepare_only=True,
                                          sem=lane_sems[prep_count[0]
                                                        % len(lane_sems)])
                                prep_count[0] += 1
                            elif GATHER_MODE == "queues":
                                kw = dict(queue_num=c + 2 * (ci % 2))
                            nc.gpsimd.dma_gather(
                                m[:].rearrange("p (t f) -> p t f", f=fo),
                                tviews[c],
                                idx_sb[c][:, ci * (CALLl // 16):
                                          (ci + 1) * (CALLl // 16)],
                                CALLl, CALLl, fo, **kw)
                            live[c][ci] = m
                            prepped[c] += 1

                    def ensure(c, t, live=live, prepped=prepped,
                               triggered=triggered, Gl=Gl, ncalls=ncalls,
                               prep=prep):
                        call = t // Gl
                        if triggered[c] <= call:
                            prep(c, call + LOOK)
                            if GATHER_MODE == "prep":
                                # one shared queue: a trigger drains BOTH
                                # streams' pending preps
                                if (prepped[0] > triggered[0]
                                        or prepped[1] > triggered[1]):
                                    nc.gpsimd.trigger_dma(count=None)
                                triggered[0] = prepped[0]
                                triggered[1] = prepped[1]
                            else:
                                triggered[c] = prepped[c]
                        return live[c][call]

                    def ensure_s4(mcol, s4_live=s4_live, s4_next=s4_next):
                        b = mcol // 4
                        while s4_next[0] <= b:
                            bi = s4_next[0]
                            m0 = bi * 4
                            nb = min(4, TP - m0)
                            s4 = sp.tile([128, 512], bf16, name="s4", tag="s4")
                            nc.vector.tensor_tensor(
                                out=s4[:].rearrange(
                                    "p (a f) -> p a f", a=4)[:, :nb, :],
                                in0=ar_sb[:].rearrange(
                                    "p (a f) -> p a f", a=4)[:, :nb, :],
                                in1=meta_sb[:, m0:m0 + nb].to_broadcast(
                                    [128, nb, 128]),
                                op=ALU.is_equal)
                            s4_live[bi] = s4
                            s4_next[0] += 1
                        return s4_live[b]

                    # desc-gen warm-up: queue LOOK+1 calls deep on both
                    # streams before the first window consumes anything.
                    prep(0, LOOK)
                    prep(1, LOOK)

                    for w in range(NW):
                        psw = ps.tile([128, fo], f32, name="psw", tag="psw")
                        first = True
                        for c, t, mcolv in window_pairs[w]:
                            m = ensure(c, t)
                            s4 = ensure_s4(mcolv)
                            j = mcolv % 4
                            sl = t % Gl
                            nc.tensor.matmul(psw[:],
                                             lhsT=s4[:, j * 128:(j + 1) * 128],
                                             rhs=m[:, sl * fo:(sl + 1) * fo],
                                             start=first, stop=False)
                            first = False
                        ob = sb.tile([128, fo], bf16, name="ob", tag="ob")
                        read_own(ob, w, li)
                        stop_here = not has_bias
                        nc.tensor.matmul(psw[:], lhsT=id_sb[:], rhs=ob[:],
                                         start=first, stop=stop_here)
                        if has_bias:
                            nc.tensor.matmul(
                                psw[:],
                                lhsT=sqd_sb[0:1, w * 128:(w + 1) * 128],
                                rhs=b_sb[li][0:1, :], start=False, stop=True)
                        if li < 2:
                            hb = sb.tile([128, fo], bf16, name="hbw",
                                         tag="hbw")
                            nc.scalar.activation(hb[:], psw[:], AF.Relu,
                                                 bias=0.0, scale=dinv_col(w))
                            gemm_block(hb, w, fo, (F2, F3)[li], w_sb[li + 1],
                                       li + 1)
                            if w == (BW if BCUT > 0 else BW - 1):
                                emit_agA(li + 1)
                        else:
                            nc.scalar.activation(
                                z_big[:, w * F3:(w + 1) * F3], psw[:],
                                AF.Relu, bias=0.0, scale=dinv_col(w))
                            nc.vector.reduce_sum(
                                zpart[:, w:w + 1],
                                z_big[:, w * F3:(w + 1) * F3], axis=X)
                    if li < 2:
                        emit_agB(li + 1)

            # ---- finals ----
            zsum = cst.tile([128, 1], f32)
            nc.vector.reduce_sum(zsum[:], zpart[:], axis=X)
            ones = cst.tile([128, 1], f32)
            nc.vector.memset(ones[:], 1.0)
            pss = ps1.tile([128, 16], f32)
            nc.tensor.matmul(pss[0:1, 0:1], lhsT=ones[:], rhs=zsum[:],
                             start=True, stop=True)
            tot_sb = cst.tile([1, 16], f32)
            nc.vector.memset(tot_sb[:], 0.0)
            nc.scalar.copy(tot_sb[0:1, 0:1], pss[0:1, 0:1])
            ar_in = dram.tile([1, 16], f32)
            ar_out = dram.tile([1, 16], f32, addr_space="Shared")
            nc.sync.dma_start(ar_in[:], tot_sb[:])
            nc.gpsimd.collective_compute(
                "AllReduce", ALU.add, replica_groups=RG,
                ins=[ar_in[:].opt()], outs=[ar_out[:].opt()])
            tot2 = cst.tile([1, 16], f32)
            nc.sync.dma_start(tot2[:], ar_out[:])
            tot_bc = cst.tile([128, 1], f32)
            nc.gpsimd.partition_broadcast(tot_bc[:], tot2[0:1, 0:1],
                                          channels=128)
            inv_tot = cst.tile([128, 1], f32)
            nc.vector.reciprocal(inv_tot[:], tot_bc[:])

            # z/sum -> tanh -> ^2 ; then row L2 norm, all column-batched.
            # Ping-pong z_big <-> scr to bound SBUF: scr=tanh(z);
            # z_big=tanh^2; scr=tanh^4; reduce; scr=z_big*rinv; DMA out.
            scr = cst.tile([128, NW * F3], f32)
            nc.scalar.activation(scr[:], z_big[:], AF.Tanh, bias=0.0,
                                 scale=inv_tot[:])
            nc.scalar.square(z_big[:], scr[:])
            nc.scalar.square(scr[:], z_big[:])
            s4s = cst.tile([128, NW], f32)
            nc.vector.reduce_sum(
                s4s[:].rearrange("p w -> p w ()"),
                scr[:].rearrange("p (w f) -> p w f", w=NW), axis=X)
            nmr = cst.tile([128, NW], f32)
            nc.scalar.sqrt(nmr[:], s4s[:])
            rinv = cst.tile([128, NW], f32)
            nc.vector.reciprocal(rinv[:], nmr[:])
            nc.vector.tensor_scalar_min(rinv[:], rinv[:], 1.0 / EPS)
            nc.vector.tensor_tensor(
                out=scr[:].rearrange("p (w f) -> p w f", w=NW),
                in0=z_big[:].rearrange("p (w f) -> p w f", w=NW),
                in1=rinv[:].to_broadcast([128, NW, F3]),
                op=ALU.mult)
            # write out: full windows in one strided DMA, tail separate
            nc.sync.dma_start(
                out_t.ap()[0:(NW - 1) * 128, :].rearrange(
                    "(w p) f -> p w f", p=128),
                scr[:].rearrange("p (w f) -> p w f", w=NW)[:, 0:NW - 1, :])
            tail = OWN - (NW - 1) * 128
            nc.sync.dma_start(
                out_t.ap()[(NW - 1) * 128:OWN, :],
                scr[0:tail, (NW - 1) * F3:NW * F3])

    nc.compile()
    return nc


# --------------------------------------------------------------------------
# entry point
# --------------------------------------------------------------------------

def kernel(x, edge_index, W1, b1, W2, b2, W3, b3):
    from concourse.bass_utils import run_bass_kernel_spmd

    x = np.ascontiguousarray(np.asarray(x, dtype=np.float32))
    ei = np.asarray(edge_index)
    src = np.ascontiguousarray(ei[0]).astype(np.int64)
    dst = np.ascontiguousarray(ei[1]).astype(np.int64)
    W1 = np.ascontiguousarray(np.asarray(W1, np.float32)).astype(BF16)
    W2 = np.ascontiguousarray(np.asarray(W2, np.float32)).astype(BF16)
    W3 = np.ascontiguousarray(np.asarray(W3, np.float32)).astype(BF16)
    b1 = np.asarray(b1, np.float32)
    b2 = np.asarray(b2, np.float32)
    b3 = np.asarray(b3, np.float32)
    has_bias = bool(np.any(b1) or np.any(b2) or np.any(b3))

    deg = (np.bincount(dst, minlength=N) + 1.0).astype(np.float32)
    dinv = (1.0 / np.sqrt(deg.astype(np.float64))).astype(np.float32)

    ck = hash((src.tobytes(), dst.tobytes(), has_bias))
    if ck in _BUILD_CACHE:
        nc, sched, per_core = _BUILD_CACHE[ck]
    else:
        sched, per_core = _build_schedule(src, dst)
        nc = _build_nc(sched, has_bias)
        _BUILD_CACHE[ck] = (nc, sched, per_core)

    arange4 = np.tile(np.arange(128, dtype=np.float32)[None, :],
                      (128, 4)).astype(BF16)
    ident = np.eye(128, dtype=np.float32).astype(BF16)
    in_maps = []
    for c in range(NCORES):
        lo = c * OWN
        xT_own = np.zeros((F0, OWN_PAD), BF16)
        xT_own[:, :OWN] = x[lo:lo + OWN].T.astype(BF16)
        dv = np.ones(OWN_PAD, np.float32)
        dv[:OWN] = dinv[lo:lo + OWN]
        dinv_img = np.ascontiguousarray(dv.reshape(NW, 128).T)
        m = {
            "xT_own": xT_own,
            "idxA": per_core[c]["idxA"],
            "idxB": per_core[c]["idxB"],
            "meta": per_core[c]["meta"],
            "dinv_img": dinv_img,
            "arange4": arange4,
            "ident": ident,
            "W1": W1, "W2": W2, "W3": W3,
        }
        if has_bias:
            sq = np.zeros((1, OWN_PAD), np.float32)
            sq[0, :OWN] = np.sqrt(deg[lo:lo + OWN])
            m["b1"] = b1.reshape(1, F1).astype(BF16)
            m["b2"] = b2.reshape(1, F2).astype(BF16)
            m["b3"] = b3.reshape(1, F3).astype(BF16)
            m["sqrtdeg"] = sq.astype(BF16)
        in_maps.append(m)

    res = run_bass_kernel_spmd(nc, in_maps, core_ids=list(range(NCORES)),
                               **_RUN_KWARGS)
    global _LAST
    _LAST = res
    out = np.concatenate([res.results[c]["out"] for c in range(NCORES)], axis=0)
    return out


# test.py hooks (harness never touches these)
_RUN_KWARGS = {}
_LAST = None
